# revision 1
# baseline (speedup 1.0000x reference)
"""Trainium2 Bass kernel for nn_DetectionLoss (B=16, N=25000, M=64).

Strategy (validated against the reference in numpy):
- Data-parallel: 8 cores x 2 images each. Host shards batch, kernel returns
  per-image losses [2] per core, host averages 16 values.
- The reference's sequential greedy match is argmax-parallel: idx[j] =
  argmax_n iou(n, j) independently per GT, and acceptance ok[j] is
  "thr[j] and no earlier thr-true GT shares idx[j]" (exact reformulation).
- Ranking uses q = inter/(area_p+area_t); iou = q/(1-q) is monotone in q, so
  argmax q == argmax iou and (iou > 0.2) == (q > 1/6). f32-exact on the data.
- Pred-partition layout: pred n lives at (partition p, slot c), n = p*196 + c.
  Pairwise ops are stride-0-AP tensor_tensor over [128, 64(gt), U(slots)].
- Per-GT argmax: reduce over slots -> [128, 64]; PE transpose -> [64, 128];
  max/max_index over partitions -> p*; indirect-DMA gather of row (p*, gt)
  from a DRAM copy of q -> max_index over slots -> c*.
- Tail (dedup, conf/box losses) on 64 partitions with tiny ops; focal's
  sigmoid/softplus via the Exp table + Newton log, arctan via polynomial
  (this neuronxcc build has no Sigmoid-free Softplus/Arctan/Ln tables).
"""

import numpy as np

B, N, M = 16, 25000, 64
P = 128            # SBUF partitions; pred partition layout
SLOTS = 196        # slots per partition; P*SLOTS = 25088 >= N
IMGS_PER_CORE = 2
N_CORES = 8
UG = 14            # slots per bulk group
NGROUPS = SLOTS // UG  # 14

# partition 0..126 have all SLOTS valid; partition 127 has PAD_START.. invalid
PAD_PART = 127
PAD_START = N - PAD_PART * SLOTS   # 25000 - 24892 = 108

_cache = {}


def _build(debug_dumps=False):
    import concourse.bass as bass
    import concourse.bacc as bacc
    import concourse.mybir as mybir
    from concourse import tile
    from concourse.bass import IndirectOffsetOnAxis
    from concourse.masks import make_identity

    f32 = mybir.dt.float32
    u32 = mybir.dt.uint32
    i32 = mybir.dt.int32
    Alu = mybir.AluOpType
    Act = mybir.ActivationFunctionType
    X = mybir.AxisListType.X
    C = mybir.AxisListType.C

    nc = bacc.Bacc("TRN2", target_bir_lowering=False, debug=False,
                   num_devices=N_CORES)

    preds_d = nc.dram_tensor("preds", [IMGS_PER_CORE, N, 5], f32, kind="ExternalInput")
    targets_d = nc.dram_tensor("targets", [IMGS_PER_CORE, M, 4], f32, kind="ExternalInput")
    out_d = nc.dram_tensor("out", [IMGS_PER_CORE], f32, kind="ExternalOutput")
    # scratch DRAM: q matrix per image, and tiny bounce buffers
    q_d = [nc.dram_tensor(f"q_scratch{b}", [P * M, SLOTS], f32)
           for b in range(IMGS_PER_CORE)]

    EPS = np.float32(1e-7)
    C_4PI2 = np.float32(4.0 / (np.pi ** 2))
    # ln(1+u)/u seed polynomial (u in (0,1]), high->low degree
    SP_SEED = [0.041064513, -0.156028432, 0.304672365, -0.496368282, 0.999887926]
    # atan(r)/r polynomial in r^2 (r in [0,1]), high->low degree
    AT_POLY = [0.0030496317, -0.0168262157, 0.0438537714, -0.0759666934,
               0.1068136135, -0.1421318243, 0.1999371457, -0.3333312071,
               0.9999999881]

    with tile.TileContext(nc) as tc:
        with (
            tc.tile_pool(name="qpool", bufs=1) as big,      # q matrix (49KB/part)
            tc.tile_pool(name="ppool", bufs=2) as ppool,    # predsI
            tc.tile_pool(name="der", bufs=2) as der,        # derived pred tiles
            tc.tile_pool(name="grp", bufs=2) as grp,        # bulk group temps
            tc.tile_pool(name="sml", bufs=2) as sml,        # small/tail temps
            tc.tile_pool(name="cst", bufs=1) as cst,        # constants
            tc.tile_pool(name="psum", bufs=2,
                         space=bass.MemorySpace.PSUM) as psum,
        ):
            # constant iotas for the tail
            iota_p64 = cst.tile([M, 1], i32, tag="iota_p64")
            nc.gpsimd.iota(iota_p64[:], pattern=[[1, 1]], base=0, channel_multiplier=1)
            iota_f64 = cst.tile([M, M], i32, tag="iota_f64")
            nc.gpsimd.iota(iota_f64[:], pattern=[[1, M]], base=0, channel_multiplier=0)
            iota_p64f = cst.tile([M, 1], f32, tag="iota_p64f")
            nc.vector.tensor_copy(iota_p64f[:], iota_p64[:])
            iota_f64f = cst.tile([M, M], f32, tag="iota_f64f")
            nc.vector.tensor_copy(iota_f64f[:], iota_f64[:])
            # lt[j, j'] = 1.0 if j' < j  (f32)
            ltmask = cst.tile([M, M], f32, tag="ltmask")
            nc.vector.tensor_scalar(ltmask[:], iota_f64f[:], iota_p64f[:], None,
                                    op0=Alu.is_lt)
            ones_row = cst.tile([1, P], f32, tag="ones_row")
            nc.gpsimd.memset(ones_row[:], 1.0)
            ident = cst.tile([P, P], f32, tag="ident")
            make_identity(nc, ident[:])

            def mkdbg(b):
                def dbg(name, ap, shape, dtype=f32):
                    if not debug_dumps:
                        return
                    t = nc.dram_tensor(f"dbg_{name}_{b}", shape, dtype,
                                       kind="ExternalOutput")
                    nc.sync.dma_start(t.ap(), ap)
                return dbg

            for b in range(IMGS_PER_CORE):
                dbg = mkdbg(b)
                # ---------------- load preds ----------------
                predsI = ppool.tile([P, SLOTS, 5], f32, tag="predsI")
                # pad defaults first (partition 127, slots >= PAD_START keep
                # them; engines can't address a partition-127 slice, so memset
                # all partitions and let the loads overwrite the valid ones):
                # boxes far away, conf = -80 (focal0 underflows to 0)
                nc.gpsimd.memset(predsI[:, PAD_START:, 0:2], 50.0)
                nc.gpsimd.memset(predsI[:, PAD_START:, 2:4], 1e-4)
                nc.gpsimd.memset(predsI[:, PAD_START:, 4:5], -80.0)
                src = preds_d.ap()[b].rearrange("n c -> (n c)")
                nc.sync.dma_start(
                    predsI[:PAD_PART],
                    src[: PAD_PART * SLOTS * 5].rearrange("(p f) -> p f", p=PAD_PART)
                    .rearrange("p (s c) -> p s c", c=5))
                nc.sync.dma_start(
                    predsI[PAD_PART:, :PAD_START],
                    src[PAD_PART * SLOTS * 5:].rearrange("(p s c) -> p s c", p=1, c=5))

                # ---------------- derived pred tiles [P, SLOTS] ----------------
                wc = der.tile([P, SLOTS], f32, tag="wc")
                hc = der.tile([P, SLOTS], f32, tag="hc")
                x1p = der.tile([P, SLOTS], f32, tag="x1p")
                x2p = der.tile([P, SLOTS], f32, tag="x2p")
                y1p = der.tile([P, SLOTS], f32, tag="y1p")
                y2p = der.tile([P, SLOTS], f32, tag="y2p")
                apred = der.tile([P, SLOTS], f32, tag="apred")
                half = der.tile([P, SLOTS], f32, tag="half")
                half2 = der.tile([P, SLOTS], f32, tag="half2")
                nc.vector.tensor_scalar_max(wc[:], predsI[:, :, 2], 1e-4)
                nc.vector.tensor_scalar_max(hc[:], predsI[:, :, 3], 1e-4)
                nc.vector.tensor_scalar_mul(half[:], wc[:], 0.5)
                nc.gpsimd.tensor_tensor(x1p[:], predsI[:, :, 0], half[:],
                                        op=Alu.subtract)
                nc.gpsimd.tensor_tensor(x2p[:], predsI[:, :, 0], half[:],
                                        op=Alu.add)
                nc.vector.tensor_scalar_mul(half2[:], hc[:], 0.5)
                nc.gpsimd.tensor_tensor(y1p[:], predsI[:, :, 1], half2[:],
                                        op=Alu.subtract)
                nc.gpsimd.tensor_tensor(y2p[:], predsI[:, :, 1], half2[:],
                                        op=Alu.add)
                nc.gpsimd.tensor_tensor(apred[:], wc[:], hc[:], op=Alu.mult)

                # ---------------- target tiles ----------------
                # per-GT layout [M, 4] for the tail
                tg = sml.tile([M, 4], f32, tag="tg")
                nc.sync.dma_start(tg[:], targets_d.ap()[b])
                # single-partition row of all target data + derived at row
                trow = sml.tile([1, M, 4], f32, tag="trow")
                nc.sync.dma_start(trow[:], targets_d.ap()[b].unsqueeze(0))
                atrow = sml.tile([1, M, 2], f32, tag="atrow")
                nc.vector.tensor_sub(atrow[:, :, 0], trow[:, :, 2], trow[:, :, 0])
                nc.vector.tensor_sub(atrow[:, :, 1], trow[:, :, 3], trow[:, :, 1])
                nc.vector.tensor_tensor(atrow[:, :, 0], atrow[:, :, 0],
                                        atrow[:, :, 1], op=Alu.mult)
                # broadcast tiles [P, M] per coordinate via PE rank-1 matmul
                x1tB = der.tile([P, M], f32, tag="x1tB")
                y1tB = der.tile([P, M], f32, tag="y1tB")
                x2tB = der.tile([P, M], f32, tag="x2tB")
                y2tB = der.tile([P, M], f32, tag="y2tB")
                atB = der.tile([P, M], f32, tag="atB")
                for (dst, rowap) in ((x1tB, trow[:, :, 0]), (y1tB, trow[:, :, 1]),
                                     (x2tB, trow[:, :, 2]), (y2tB, trow[:, :, 3]),
                                     (atB, atrow[:, :, 0])):
                    pt = psum.tile([P, M], f32, tag="bcast_ps", name="bcast_ps")
                    nc.tensor.matmul(pt[:], ones_row[:], rowap, start=True,
                                     stop=True)
                    nc.scalar.copy(dst[:], pt[:])
                dbg("atB", atB[:], [P, M])
                dbg("x1p", x1p[:], [P, SLOTS])
                dbg("apred", apred[:], [P, SLOTS])

                # ---------------- bulk pairwise q ----------------
                q = big.tile([P, M, SLOTS], f32, tag="q")

                def pv(t, g):  # pred-derived operand [P, M, UG] (gt-major, stride0 on gt)
                    return t[:, g * UG:(g + 1) * UG].unsqueeze(1).to_broadcast([P, M, UG])

                def tv(t):     # target-broadcast operand [P, M, UG] (stride0 on slots)
                    return t[:].unsqueeze(2).to_broadcast([P, M, UG])

                for g in range(NGROUPS):
                    ltx = grp.tile([P, M, UG], f32, tag="ltx")
                    rbx = grp.tile([P, M, UG], f32, tag="rbx")
                    lty = grp.tile([P, M, UG], f32, tag="lty")
                    rby = grp.tile([P, M, UG], f32, tag="rby")
                    ssum = grp.tile([P, M, UG], f32, tag="ssum")
                    rsc = grp.tile([P, M, UG], f32, tag="rsc")
                    qslice = q[:, :, g * UG:(g + 1) * UG]

                    # GpSimd ucode only supports add/sub/mult TensorTensor
                    # (walrus rejects min/max on Pool), so min/max stay on DVE
                    # and GpSimd takes a sub and the two mults.
                    nc.vector.tensor_tensor(ltx[:], pv(x1p, g), tv(x1tB), op=Alu.max)
                    nc.vector.tensor_tensor(rbx[:], pv(x2p, g), tv(x2tB), op=Alu.min)
                    nc.vector.tensor_tensor(lty[:], pv(y1p, g), tv(y1tB), op=Alu.max)
                    nc.vector.tensor_tensor(rby[:], pv(y2p, g), tv(y2tB), op=Alu.min)
                    nc.gpsimd.tensor_tensor(rbx[:], rbx[:], ltx[:], op=Alu.subtract)
                    nc.vector.tensor_tensor(rby[:], rby[:], lty[:], op=Alu.subtract)
                    nc.scalar.activation(rbx[:], rbx[:], Act.Relu)
                    nc.scalar.activation(rby[:], rby[:], Act.Relu)
                    nc.gpsimd.tensor_tensor(rbx[:], rbx[:], rby[:], op=Alu.mult)
                    nc.vector.tensor_tensor(ssum[:], pv(apred, g), tv(atB), op=Alu.add)
                    # 51-ULP approx reciprocal, 1 op, ~5x faster than
                    # reciprocal(); validated argmax/thr-safe on the data
                    # (worst q rel err 3.3e-6 vs 1.26e-5 min top-2 gap)
                    nc.vector.reciprocal_approx_fast(rsc[:], ssum[:])
                    nc.gpsimd.tensor_tensor(qslice, rbx[:], rsc[:], op=Alu.mult)

                # ship q to DRAM for the later row gather
                nc.sync.dma_start(q_d[b].ap().rearrange("(p m) s -> p m s", p=P), q[:])
                m1 = sml.tile([P, M], f32, tag="m1")
                nc.vector.tensor_reduce(m1[:], q[:], axis=X, op=Alu.max)
                dbg("m1", m1[:], [P, M])
                dbg("q0", q[:, 0:2], [P, 2, SLOTS])
                # transpose m1 on PE -> [M, P]
                m1tp = psum.tile([M, P], f32, tag="m1tp", name="m1tp")
                nc.tensor.transpose(m1tp[:], m1[:], ident[:])
                m1t = sml.tile([M, P], f32, tag="m1t")
                nc.vector.tensor_copy(m1t[:], m1tp[:])
                # top-1 over partitions per GT
                mx8 = sml.tile([M, 8], f32, tag="mx8")
                pi8 = sml.tile([M, 8], u32, tag="pi8")
                nc.vector.max(mx8[:], m1t[:])
                nc.vector.max_index(pi8[:], mx8[:], m1t[:])
                dbg("m1t", m1t[:], [M, P])
                dbg("mx8", mx8[:], [M, 8])
                dbg("pi8", pi8[:], [M, 8], u32)

                # ---------------- level-2: recover slot via row gather ----------
                rowoff = sml.tile([M, 2], u32, tag="rowoff")
                nc.vector.tensor_scalar_mul(rowoff[:, 0:1], pi8[:, 0:1], M)
                nc.vector.tensor_tensor(rowoff[:, 0:1], rowoff[:, 0:1],
                                        iota_p64[:].bitcast(u32), op=Alu.add)
                qrow = sml.tile([M, SLOTS], f32, tag="qrow")
                nc.gpsimd.indirect_dma_start(
                    out=qrow[:], out_offset=None,
                    in_=q_d[b].ap(),
                    in_offset=IndirectOffsetOnAxis(ap=rowoff[:, 0:1], axis=0))
                qx8 = sml.tile([M, 8], f32, tag="qx8")
                ci8 = sml.tile([M, 8], u32, tag="ci8")
                nc.vector.max(qx8[:], qrow[:])
                nc.vector.max_index(ci8[:], qx8[:], qrow[:])
                dbg("qrow", qrow[:], [M, SLOTS])
                dbg("qx8", qx8[:], [M, 8])
                dbg("ci8", ci8[:], [M, 8], u32)

                # n* = p* * SLOTS + c*  (u32), maxq = qx8[:,0:1]
                nstar = sml.tile([M, 1], u32, tag="nstar")
                nc.vector.tensor_scalar_mul(nstar[:], pi8[:, 0:1], SLOTS)
                nc.vector.tensor_tensor(nstar[:], nstar[:], ci8[:, 0:1], op=Alu.add)
                maxq = qx8[:, 0:1]

                # thr = maxq > 1/6
                thr = sml.tile([M, 1], f32, tag="thr")
                nc.vector.tensor_scalar(thr[:], maxq, float(1.0 / 6.0), None,
                                        op0=Alu.is_gt)

                # ---------------- dedup: ok[j] = thr[j] & !any(j'<j, thr & same n*) --
                nstar_f = sml.tile([M, 1], f32, tag="nstar_f")
                nc.vector.tensor_copy(nstar_f[:], nstar[:])  # u32 -> f32 convert
                # transpose (n*, thr) to a row on PE, broadcast over partitions
                pair = sml.tile([M, 2], f32, tag="pair")
                nc.vector.tensor_copy(pair[:, 0:1], nstar_f[:])
                nc.vector.tensor_copy(pair[:, 1:2], thr[:])
                pairT_ps = psum.tile([1, 2, M], f32, tag="pairT_ps", name="pairT_ps")
                nc.tensor.transpose(pairT_ps[:, 0], pair[:, 0:1], ident[:M, :M])
                nc.tensor.transpose(pairT_ps[:, 1], pair[:, 1:2], ident[:M, :M])
                pairT = sml.tile([1, 2, M], f32, tag="pairT")
                nc.vector.tensor_copy(pairT[:], pairT_ps[:])
                rowB = sml.tile([M, M, 2], f32, tag="rowB")
                ptb = psum.tile([M, M, 2], f32, tag="ptb", name="ptb")
                nc.tensor.matmul(ptb[:, :, 0], ones_row[:, :M], pairT[:, 0],
                                 start=True, stop=True)
                nc.tensor.matmul(ptb[:, :, 1], ones_row[:, :M], pairT[:, 1],
                                 start=True, stop=True)
                nc.scalar.copy(rowB[:], ptb[:])
                eq = sml.tile([M, M], f32, tag="eq")
                nc.vector.tensor_scalar(eq[:], rowB[:, :, 0], nstar_f[:], None,
                                        op0=Alu.is_equal)
                nc.gpsimd.tensor_tensor(eq[:], eq[:], rowB[:, :, 1], op=Alu.mult)
                nc.vector.tensor_tensor(eq[:], eq[:], ltmask[:], op=Alu.mult)
                blocked = sml.tile([M, 1], f32, tag="blocked")
                nc.vector.tensor_reduce(blocked[:], eq[:], axis=X, op=Alu.max)
                ok = sml.tile([M, 1], f32, tag="ok")
                nc.vector.tensor_scalar(ok[:], blocked[:], -1.0, 1.0,
                                        op0=Alu.mult, op1=Alu.add)
                nc.gpsimd.tensor_tensor(ok[:], ok[:], thr[:], op=Alu.mult)
                dbg("nstar", nstar[:], [M, 1], u32)
                dbg("thr", thr[:], [M, 1])
                dbg("ok", ok[:], [M, 1])

                # ---------------- gather matched preds [M, 5] ----------------
                g5 = sml.tile([M, 5], f32, tag="g5")
                nrow = sml.tile([M, 1], u32, tag="nrow")
                nc.vector.tensor_scalar_add(nrow[:], nstar[:], b * N)
                nc.gpsimd.indirect_dma_start(
                    out=g5[:], out_offset=None,
                    in_=preds_d.ap().rearrange("b n c -> (b n) c"),
                    in_offset=IndirectOffsetOnAxis(ap=nrow[:], axis=0))
                dbg("g5", g5[:], [M, 5])

                # ---------------- ciou on [M, 1] ----------------
                t1 = lambda tag: sml.tile([M, 1], f32, tag=tag, name=tag)
                gwc, ghc, gh2 = t1("gwc"), t1("ghc"), t1("gh2")
                nc.vector.tensor_scalar_max(gwc[:], g5[:, 2:3], 1e-4)
                nc.vector.tensor_scalar_max(ghc[:], g5[:, 3:4], 1e-4)
                px1, px2, py1, py2 = t1("px1"), t1("px2"), t1("py1"), t1("py2")
                nc.vector.tensor_scalar_mul(gh2[:], gwc[:], 0.5)
                nc.vector.tensor_sub(px1[:], g5[:, 0:1], gh2[:])
                nc.vector.tensor_add(px2[:], g5[:, 0:1], gh2[:])
                nc.vector.tensor_scalar_mul(gh2[:], ghc[:], 0.5)
                nc.vector.tensor_sub(py1[:], g5[:, 1:2], gh2[:])
                nc.vector.tensor_add(py2[:], g5[:, 1:2], gh2[:])
                tx1, ty1, tx2, ty2 = tg[:, 0:1], tg[:, 1:2], tg[:, 2:3], tg[:, 3:4]

                a1, a2, a3, a4 = t1("a1"), t1("a2"), t1("a3"), t1("a4")
                # inter
                nc.vector.tensor_tensor(a1[:], px1[:], tx1, op=Alu.max)
                nc.vector.tensor_tensor(a2[:], px2[:], tx2, op=Alu.min)
                nc.vector.tensor_sub(a2[:], a2[:], a1[:])
                nc.vector.tensor_scalar_max(a2[:], a2[:], 0.0)
                nc.vector.tensor_tensor(a3[:], py1[:], ty1, op=Alu.max)
                nc.vector.tensor_tensor(a4[:], py2[:], ty2, op=Alu.min)
                nc.vector.tensor_sub(a4[:], a4[:], a3[:])
                nc.vector.tensor_scalar_max(a4[:], a4[:], 0.0)
                ginter = t1("ginter")
                nc.vector.tensor_tensor(ginter[:], a2[:], a4[:], op=Alu.mult)
                # union = ap + at - inter  (areas from xyxy, matching reference)
                gwp, ghp, gwt, ght = t1("gwp"), t1("ghp"), t1("gwt"), t1("ght")
                nc.vector.tensor_sub(gwp[:], px2[:], px1[:])
                nc.vector.tensor_sub(ghp[:], py2[:], py1[:])
                nc.vector.tensor_sub(gwt[:], tx2, tx1)
                nc.vector.tensor_sub(ght[:], ty2, ty1)
                gu = t1("gu")
                nc.vector.tensor_tensor(gu[:], gwp[:], ghp[:], op=Alu.mult)
                nc.vector.tensor_tensor(a1[:], gwt[:], ght[:], op=Alu.mult)
                nc.vector.tensor_add(gu[:], gu[:], a1[:])
                nc.vector.tensor_sub(gu[:], gu[:], ginter[:])
                giou = t1("giou")
                nc.vector.tensor_scalar_add(gu[:], gu[:], float(EPS))
                nc.vector.reciprocal(gu[:], gu[:])
                nc.vector.tensor_tensor(giou[:], ginter[:], gu[:], op=Alu.mult)
                # enclosing box diag
                nc.vector.tensor_tensor(a1[:], px1[:], tx1, op=Alu.min)
                nc.vector.tensor_tensor(a2[:], px2[:], tx2, op=Alu.max)
                nc.vector.tensor_sub(a2[:], a2[:], a1[:])
                nc.vector.tensor_tensor(a2[:], a2[:], a2[:], op=Alu.mult)
                nc.vector.tensor_tensor(a3[:], py1[:], ty1, op=Alu.min)
                nc.vector.tensor_tensor(a4[:], py2[:], ty2, op=Alu.max)
                nc.vector.tensor_sub(a4[:], a4[:], a3[:])
                nc.vector.tensor_tensor(a4[:], a4[:], a4[:], op=Alu.mult)
                diag = t1("diag")
                nc.vector.tensor_add(diag[:], a2[:], a4[:])
                nc.vector.tensor_scalar_add(diag[:], diag[:], float(EPS))
                # center distance term
                nc.vector.tensor_add(a1[:], px1[:], px2[:])
                nc.vector.tensor_sub(a1[:], a1[:], tx1)
                nc.vector.tensor_sub(a1[:], a1[:], tx2)
                nc.vector.tensor_tensor(a1[:], a1[:], a1[:], op=Alu.mult)
                nc.vector.tensor_add(a3[:], py1[:], py2[:])
                nc.vector.tensor_sub(a3[:], a3[:], ty1)
                nc.vector.tensor_sub(a3[:], a3[:], ty2)
                nc.vector.tensor_tensor(a3[:], a3[:], a3[:], op=Alu.mult)
                cent = t1("cent")
                nc.vector.tensor_add(cent[:], a1[:], a3[:])
                nc.vector.tensor_scalar_mul(cent[:], cent[:], 0.25)
                # diou = 1 - iou + cent/diag
                diou = t1("diou")
                nc.vector.reciprocal(diag[:], diag[:])
                nc.vector.tensor_tensor(diou[:], cent[:], diag[:], op=Alu.mult)
                nc.vector.tensor_sub(diou[:], diou[:], giou[:])
                nc.vector.tensor_scalar_add(diou[:], diou[:], 1.0)
                # v = 4/pi^2 * (atan(wt/ht) - atan(wp/hp))^2
                # atan via odd polynomial + inversion (no Arctan table on HW)
                vv = t1("vv")
                rat = sml.tile([M, 2], f32, tag="rat", name="rat")
                big2 = sml.tile([M, 2], i32, tag="big2", name="big2")
                inv2 = sml.tile([M, 2], f32, tag="inv2", name="inv2")
                s2 = sml.tile([M, 2], f32, tag="s2", name="s2")
                ac2 = sml.tile([M, 2], f32, tag="ac2", name="ac2")
                nc.vector.reciprocal(rat[:, 0:1], ght[:])
                nc.vector.tensor_tensor(rat[:, 0:1], gwt[:], rat[:, 0:1], op=Alu.mult)
                nc.vector.reciprocal(rat[:, 1:2], ghp[:])
                nc.vector.tensor_tensor(rat[:, 1:2], gwp[:], rat[:, 1:2], op=Alu.mult)
                nc.vector.tensor_scalar(big2[:], rat[:], 1.0, None, op0=Alu.is_gt)
                nc.vector.reciprocal(inv2[:], rat[:])
                nc.vector.copy_predicated(rat[:], big2[:], inv2[:])
                nc.vector.tensor_tensor(s2[:], rat[:], rat[:], op=Alu.mult)
                nc.vector.tensor_scalar(ac2[:], s2[:], float(AT_POLY[0]),
                                        float(AT_POLY[1]), op0=Alu.mult, op1=Alu.add)
                for coef in AT_POLY[2:]:
                    nc.vector.tensor_tensor(ac2[:], ac2[:], s2[:], op=Alu.mult)
                    nc.vector.tensor_scalar_add(ac2[:], ac2[:], float(coef))
                nc.vector.tensor_tensor(ac2[:], ac2[:], rat[:], op=Alu.mult)
                nc.vector.tensor_scalar(inv2[:], ac2[:], -1.0, float(np.pi / 2),
                                        op0=Alu.mult, op1=Alu.add)
                nc.vector.copy_predicated(ac2[:], big2[:], inv2[:])
                nc.vector.tensor_sub(vv[:], ac2[:, 0:1], ac2[:, 1:2])
                nc.vector.tensor_tensor(vv[:], vv[:], vv[:], op=Alu.mult)
                nc.vector.tensor_scalar_mul(vv[:], vv[:], float(C_4PI2))
                # alpha = v / (1 - iou + v + eps)
                nc.vector.tensor_scalar(a1[:], giou[:], -1.0, float(1.0 + EPS),
                                        op0=Alu.mult, op1=Alu.add)
                nc.vector.tensor_add(a1[:], a1[:], vv[:])
                nc.vector.reciprocal(a1[:], a1[:])
                nc.vector.tensor_tensor(a1[:], a1[:], vv[:], op=Alu.mult)
                ciou = t1("ciou")
                nc.vector.tensor_tensor(ciou[:], a1[:], vv[:], op=Alu.mult)
                nc.vector.tensor_add(ciou[:], ciou[:], diou[:])
                dbg("ciou", ciou[:], [M, 1])
                # box_loss = sum(ciou*ok)/max(n_match,1)
                nc.vector.tensor_tensor(a1[:], ciou[:], ok[:], op=Alu.mult)
                bsum = sml.tile([1, 1], f32, tag="bsum")
                nmatch = sml.tile([1, 1], f32, tag="nmatch")
                nc.gpsimd.tensor_reduce(bsum[:], a1[:], axis=C, op=Alu.add)
                nc.gpsimd.tensor_reduce(nmatch[:], ok[:], axis=C, op=Alu.add)
                dbg("nmraw", nmatch[:], [1, 1])
                dbg("ok2", ok[:], [M, 1])
                nc.vector.tensor_scalar_max(nmatch[:], nmatch[:], 1.0)
                nc.vector.reciprocal(nmatch[:], nmatch[:])
                box_loss = sml.tile([1, 1], f32, tag="box_loss")
                nc.vector.tensor_tensor(box_loss[:], bsum[:], nmatch[:], op=Alu.mult)
                dbg("nmrecip", nmatch[:], [1, 1])
                dbg("boxloss", box_loss[:], [1, 1])

                # ---------------- focal loss ----------------
                # sigmoid/softplus via Exp table + DVE (no Sigmoid/Softplus
                # table thrash; softplus = relu(x) + ln(1+exp(-|x|)) with a
                # polynomial seed + 2 Newton iterations for the log).
                def softplus_sigmoid(x_ap, shape, pool, pfx):
                    tl = lambda t: pool.tile(shape, f32, tag=pfx + t, name=pfx + t)
                    sg_, sp_, u_, w_, z_, e_ = (tl("sg"), tl("sp"), tl("u"),
                                                tl("w"), tl("z"), tl("e"))
                    # sigmoid = 1/(1+exp(-x))
                    nc.scalar.activation(e_[:], x_ap, Act.Exp, scale=-1.0)
                    nc.vector.tensor_scalar_add(e_[:], e_[:], 1.0)
                    nc.vector.reciprocal(sg_[:], e_[:])
                    # u = exp(-|x|), w = 1+u   (|x| = max(x, -x))
                    nc.vector.tensor_scalar_mul(u_[:], x_ap, -1.0)
                    nc.vector.tensor_tensor(u_[:], u_[:], x_ap, op=Alu.max)
                    nc.scalar.activation(u_[:], u_[:], Act.Exp, scale=-1.0)
                    nc.vector.tensor_scalar_add(w_[:], u_[:], 1.0)
                    # z seed = u*poly(u)
                    nc.vector.tensor_scalar(z_[:], u_[:], float(SP_SEED[0]),
                                            float(SP_SEED[1]), op0=Alu.mult,
                                            op1=Alu.add)
                    for coef in SP_SEED[2:]:
                        nc.vector.tensor_tensor(z_[:], z_[:], u_[:], op=Alu.mult)
                        nc.vector.tensor_scalar_add(z_[:], z_[:], float(coef))
                    nc.vector.tensor_tensor(z_[:], z_[:], u_[:], op=Alu.mult)
                    # 2 Newton iterations: z += w*exp(-z) - 1
                    for _ in range(2):
                        nc.scalar.activation(e_[:], z_[:], Act.Exp, scale=-1.0)
                        nc.gpsimd.tensor_tensor(e_[:], w_[:], e_[:], op=Alu.mult)
                        nc.gpsimd.tensor_tensor(z_[:], z_[:], e_[:], op=Alu.add)
                        nc.vector.tensor_scalar_add(z_[:], z_[:], -1.0)
                    # softplus = relu(x) + z
                    nc.scalar.activation(sp_[:], x_ap, Act.Relu)
                    nc.vector.tensor_add(sp_[:], sp_[:], z_[:])
                    return sg_, sp_

                conf = predsI[:, :, 4]
                sg, sp = softplus_sigmoid(conf, [P, SLOTS], der, "fb")
                f0 = der.tile([P, SLOTS], f32, tag="f0")
                nc.gpsimd.tensor_tensor(f0[:], sg[:], sg[:], op=Alu.mult)
                nc.gpsimd.tensor_tensor(f0[:], f0[:], sp[:], op=Alu.mult)
                frow = sml.tile([P, 1], f32, tag="frow")
                nc.vector.tensor_reduce(frow[:], f0[:], axis=X, op=Alu.add)
                fsum = sml.tile([1, 1], f32, tag="fsum")
                nc.gpsimd.tensor_reduce(fsum[:], frow[:], axis=C, op=Alu.add)
                dbg("fsum", fsum[:], [1, 1])
                # correction at matched preds: sum ok * (focal1 - focal0)
                xm = g5[:, 4:5]
                msg, msp = softplus_sigmoid(xm, [M, 1], sml, "fm")
                msn = t1("msn")
                # softplus(-x) = softplus(x) - x
                nc.vector.tensor_sub(msn[:], msp[:], xm)
                mf0, mf1 = t1("mf0"), t1("mf1")
                nc.vector.tensor_tensor(mf0[:], msg[:], msg[:], op=Alu.mult)
                nc.vector.tensor_tensor(mf0[:], mf0[:], msp[:], op=Alu.mult)
                nc.vector.tensor_scalar_mul(mf0[:], mf0[:], 0.75)
                nc.vector.tensor_scalar(mf1[:], msg[:], -1.0, 1.0,
                                        op0=Alu.mult, op1=Alu.add)
                nc.vector.tensor_tensor(mf1[:], mf1[:], mf1[:], op=Alu.mult)
                nc.vector.tensor_tensor(mf1[:], mf1[:], msn[:], op=Alu.mult)
                nc.vector.tensor_scalar_mul(mf1[:], mf1[:], 0.25)
                nc.vector.tensor_sub(mf1[:], mf1[:], mf0[:])
                nc.vector.tensor_tensor(mf1[:], mf1[:], ok[:], op=Alu.mult)
                dsum = sml.tile([1, 1], f32, tag="dsum")
                nc.gpsimd.tensor_reduce(dsum[:], mf1[:], axis=C, op=Alu.add)
                dbg("dsum", dsum[:], [1, 1])
                dbg("bsum", bsum[:], [1, 1])

                # per_image = (0.75*fsum + dsum)/N + box_loss
                acc = sml.tile([1, 1], f32, tag="acc")
                nc.vector.tensor_scalar_mul(acc[:], fsum[:], 0.75)
                nc.vector.tensor_add(acc[:], acc[:], dsum[:])
                nc.vector.tensor_scalar_mul(acc[:], acc[:], float(1.0 / N))
                nc.vector.tensor_add(acc[:], acc[:], box_loss[:])
                dbg("acc", acc[:], [1, 1])
                nc.sync.dma_start(out_d.ap()[b:b + 1], acc[:].rearrange("o m -> (o m)"))

    nc.compile()
    return nc


def _get_nc():
    if "nc" not in _cache:
        _cache["nc"] = _build()
    return _cache["nc"]


def kernel(preds: np.ndarray, targets: np.ndarray) -> np.ndarray:
    from concourse.bass_utils import run_bass_kernel_spmd

    nc = _get_nc()
    preds = np.ascontiguousarray(preds, dtype=np.float32)
    targets = np.ascontiguousarray(targets, dtype=np.float32)
    in_maps = []
    for c in range(N_CORES):
        s = c * IMGS_PER_CORE
        in_maps.append({"preds": preds[s:s + IMGS_PER_CORE],
                        "targets": targets[s:s + IMGS_PER_CORE]})
    res = run_bass_kernel_spmd(nc, in_maps, list(range(N_CORES)))
    per_image = np.concatenate([res.results[c]["out"] for c in range(N_CORES)])
    return np.float32(per_image.mean())



# revision 3
# speedup vs baseline: 1.3383x; 1.3383x over previous
"""Trainium2 Bass kernel for nn_DetectionLoss (B=16, N=25000, M=64).

Strategy (v2 — fp16 match pipeline):
- Data-parallel: 8 cores x 2 images each. Host shards batch, kernel returns
  per-image losses [2] per core, host averages 16 values.
- Greedy match reformulated as per-GT argmax (exact, see baseline notes).
  Ranking uses q = inter/(area_p+area_t), monotone in iou; thr is q > 1/6.
- The match DECISIONS tolerate fp16: numpy sim shows 47/1024 idx flips and
  1/1024 mask flips -> final rel err 3.5e-4 (gate is 2e-2). The loss tail
  (ciou/focal at matched GTs) stays exact f32 via DRAM gathers.
- Bulk pairwise runs in fp16 to hit the DVE 2x_1p mode (527ns vs 994ns per
  [128,64x28] op). Broadcast target operands would kill the mode (stride-0
  last dim), so each target channel is materialized once per image as a
  [P, M, UG] replicated tile (log-doubling TensorCopy at 4x) and reused by
  all 7 slot groups.
- Engine balance per group: DVE does 4x minmax (f16 2x), recip (f32),
  inter/q (f16 2x), running-max accumulate; Pool does dx/dy subs + ssum
  (broadcast add); Act does relu + rsc f32->f16 convert.
- Per-GT argmax: running max macc [P,M,28] folded into the group loop;
  final reduce -> m1 [P,M] f16; PE transpose; partition argmax; indirect
  row gather from a DRAM fp16 copy of q recovers the slot index (exact
  first-occurrence tie-break in both paths = smallest n, matching jnp).
- Tail (dedup, ciou, focal) unchanged from baseline (f32, tiny tiles).
"""

import numpy as np

B, N, M = 16, 25000, 64
P = 128            # SBUF partitions; pred partition layout
SLOTS = 196        # slots per partition; P*SLOTS = 25088 >= N
IMGS_PER_CORE = 2
N_CORES = 8
UG = 28            # slots per bulk group
NGROUPS = SLOTS // UG  # 7

# partition 0..126 have all SLOTS valid; partition 127 has PAD_START.. invalid
PAD_PART = 127
PAD_START = N - PAD_PART * SLOTS   # 25000 - 24892 = 108

_cache = {}


def _build(debug_dumps=False):
    import concourse.bass as bass
    import concourse.bacc as bacc
    import concourse.mybir as mybir
    from concourse import tile
    from concourse.bass import IndirectOffsetOnAxis
    from concourse.masks import make_identity

    f32 = mybir.dt.float32
    f16 = mybir.dt.float16
    u32 = mybir.dt.uint32
    i32 = mybir.dt.int32
    Alu = mybir.AluOpType
    Act = mybir.ActivationFunctionType
    X = mybir.AxisListType.X
    C = mybir.AxisListType.C

    nc = bacc.Bacc("TRN2", target_bir_lowering=False, debug=False,
                   num_devices=N_CORES)

    preds_d = nc.dram_tensor("preds", [IMGS_PER_CORE, N, 5], f32, kind="ExternalInput")
    targets_d = nc.dram_tensor("targets", [IMGS_PER_CORE, M, 4], f32, kind="ExternalInput")
    out_d = nc.dram_tensor("out", [IMGS_PER_CORE], f32, kind="ExternalOutput")
    # scratch DRAM: fp16 q matrix per image for the later row gather
    q_d = [nc.dram_tensor(f"q_scratch{b}", [P * M, SLOTS], f16)
           for b in range(IMGS_PER_CORE)]

    EPS = np.float32(1e-7)
    C_4PI2 = np.float32(4.0 / (np.pi ** 2))
    # ln(1+u)/u seed polynomial (u in (0,1]), high->low degree
    SP_SEED = [0.041064513, -0.156028432, 0.304672365, -0.496368282, 0.999887926]
    # atan(r)/r polynomial in r^2 (r in [0,1]), high->low degree
    AT_POLY = [0.0030496317, -0.0168262157, 0.0438537714, -0.0759666934,
               0.1068136135, -0.1421318243, 0.1999371457, -0.3333312071,
               0.9999999881]

    with tile.TileContext(nc) as tc:
        with (
            tc.tile_pool(name="qpool", bufs=2) as qpool,    # q matrix f16
            tc.tile_pool(name="ppool", bufs=2) as ppool,    # predsI
            tc.tile_pool(name="der", bufs=2) as der,        # derived pred tiles
            tc.tile_pool(name="rep", bufs=2) as rep,        # replicated tgt tiles
            tc.tile_pool(name="grp", bufs=2) as grp,        # bulk group temps
            tc.tile_pool(name="mac", bufs=2) as mac,        # running max
            tc.tile_pool(name="sml", bufs=2) as sml,        # small/tail temps
            tc.tile_pool(name="cst", bufs=1) as cst,        # constants
            tc.tile_pool(name="psum", bufs=2,
                         space=bass.MemorySpace.PSUM) as psum,
        ):
            # constant iotas for the tail
            iota_p64 = cst.tile([M, 1], i32, tag="iota_p64")
            nc.gpsimd.iota(iota_p64[:], pattern=[[1, 1]], base=0, channel_multiplier=1)
            iota_f64 = cst.tile([M, M], i32, tag="iota_f64")
            nc.gpsimd.iota(iota_f64[:], pattern=[[1, M]], base=0, channel_multiplier=0)
            iota_p64f = cst.tile([M, 1], f32, tag="iota_p64f")
            nc.vector.tensor_copy(iota_p64f[:], iota_p64[:])
            iota_f64f = cst.tile([M, M], f32, tag="iota_f64f")
            nc.vector.tensor_copy(iota_f64f[:], iota_f64[:])
            # lt[j, j'] = 1.0 if j' < j  (f32)
            ltmask = cst.tile([M, M], f32, tag="ltmask")
            nc.vector.tensor_scalar(ltmask[:], iota_f64f[:], iota_p64f[:], None,
                                    op0=Alu.is_lt)
            ones_row = cst.tile([1, P], f32, tag="ones_row")
            nc.gpsimd.memset(ones_row[:], 1.0)
            ident = cst.tile([P, P], f32, tag="ident")
            make_identity(nc, ident[:])
            ident16 = cst.tile([P, P], f16, tag="ident16")
            nc.vector.tensor_copy(ident16[:], ident[:])

            def mkdbg(b):
                def dbg(name, ap, shape, dtype=f32):
                    if not debug_dumps:
                        return
                    t = nc.dram_tensor(f"dbg_{name}_{b}", shape, dtype,
                                       kind="ExternalOutput")
                    nc.sync.dma_start(t.ap(), ap)
                return dbg

            # ---------------- per-image prelude ----------------
            st = [dict() for _ in range(IMGS_PER_CORE)]
            for b in range(IMGS_PER_CORE):
                s = st[b]
                # load preds
                predsI = ppool.tile([P, SLOTS, 5], f32, tag="predsI", name="predsI")
                nc.gpsimd.memset(predsI[:, PAD_START:, 0:2], 50.0)
                nc.gpsimd.memset(predsI[:, PAD_START:, 2:4], 1e-4)
                nc.gpsimd.memset(predsI[:, PAD_START:, 4:5], -80.0)
                src = preds_d.ap()[b].rearrange("n c -> (n c)")
                nc.sync.dma_start(
                    predsI[:PAD_PART],
                    src[: PAD_PART * SLOTS * 5].rearrange("(p f) -> p f", p=PAD_PART)
                    .rearrange("p (s c) -> p s c", c=5))
                nc.sync.dma_start(
                    predsI[PAD_PART:, :PAD_START],
                    src[PAD_PART * SLOTS * 5:].rearrange("(p s c) -> p s c", p=1, c=5))
                s["predsI"] = predsI

                # derived pred tiles: f16 box coords + f32 area
                wc = der.tile([P, SLOTS], f32, tag="wc", name="wc")
                hc = der.tile([P, SLOTS], f32, tag="hc", name="hc")
                half = der.tile([P, SLOTS], f32, tag="half", name="half")
                x1p = der.tile([P, SLOTS], f16, tag="x1p", name="x1p")
                x2p = der.tile([P, SLOTS], f16, tag="x2p", name="x2p")
                y1p = der.tile([P, SLOTS], f16, tag="y1p", name="y1p")
                y2p = der.tile([P, SLOTS], f16, tag="y2p", name="y2p")
                apred = der.tile([P, SLOTS], f32, tag="apred", name="apred")
                nc.vector.tensor_scalar_max(wc[:], predsI[:, :, 2], 1e-4)
                nc.vector.tensor_scalar_max(hc[:], predsI[:, :, 3], 1e-4)
                nc.vector.tensor_scalar_mul(half[:], wc[:], 0.5)
                nc.vector.tensor_tensor(x1p[:], predsI[:, :, 0], half[:],
                                        op=Alu.subtract)
                nc.vector.tensor_tensor(x2p[:], predsI[:, :, 0], half[:],
                                        op=Alu.add)
                nc.vector.tensor_scalar_mul(half[:], hc[:], 0.5)
                nc.vector.tensor_tensor(y1p[:], predsI[:, :, 1], half[:],
                                        op=Alu.subtract)
                nc.vector.tensor_tensor(y2p[:], predsI[:, :, 1], half[:],
                                        op=Alu.add)
                nc.vector.tensor_tensor(apred[:], wc[:], hc[:], op=Alu.mult)
                s.update(x1p=x1p, x2p=x2p, y1p=y1p, y2p=y2p, apred=apred)

                # target tiles: [M, 4] for the tail, row + PE broadcast
                tg = sml.tile([M, 4], f32, tag="tg", name="tg")
                nc.sync.dma_start(tg[:], targets_d.ap()[b])
                trow = sml.tile([1, M, 4], f32, tag="trow", name="trow")
                nc.sync.dma_start(trow[:], targets_d.ap()[b].unsqueeze(0))
                atrow = sml.tile([1, M, 2], f32, tag="atrow", name="atrow")
                nc.vector.tensor_sub(atrow[:, :, 0], trow[:, :, 2], trow[:, :, 0])
                nc.vector.tensor_sub(atrow[:, :, 1], trow[:, :, 3], trow[:, :, 1])
                nc.vector.tensor_tensor(atrow[:, :, 0], atrow[:, :, 0],
                                        atrow[:, :, 1], op=Alu.mult)
                s["tg"] = tg

                # PE broadcast -> [P, M]: 4 coord channels f16, area f32
                coord16 = []
                for ci in range(4):
                    pt = psum.tile([P, M], f32, tag="bcast_ps", name="bcast_ps")
                    nc.tensor.matmul(pt[:], ones_row[:], trow[:, :, ci],
                                     start=True, stop=True)
                    c16 = rep.tile([P, M], f16, tag=f"tb16_{ci}", name=f"tb16_{ci}")
                    nc.scalar.copy(c16[:], pt[:])
                    coord16.append(c16)
                pt = psum.tile([P, M], f32, tag="bcast_ps", name="bcast_ps")
                nc.tensor.matmul(pt[:], ones_row[:], atrow[:, :, 0],
                                 start=True, stop=True)
                atB = rep.tile([P, M], f32, tag="atB", name="atB")
                nc.scalar.copy(atB[:], pt[:])
                s["atB"] = atB

                # replicate each f16 coord channel to [P, M, UG] (log doubling)
                reps = []
                for ci in range(4):
                    r = rep.tile([P, M, UG], f16, tag=f"rep_{ci}", name=f"rep_{ci}")
                    nc.vector.tensor_copy(r[:, :, 0:1],
                                          coord16[ci][:].unsqueeze(2))
                    k = 1
                    while k < UG:
                        step = min(k, UG - k)
                        nc.vector.tensor_copy(r[:, :, k:k + step], r[:, :, 0:step])
                        k += step
                    reps.append(r)
                s["reps"] = reps

                s["q"] = qpool.tile([P, M, SLOTS], f16, tag="q", name="q")
                s["macc"] = mac.tile([P, M, UG], f16, tag="macc", name="macc")

            # ---------------- bulk pairwise, images interleaved ----------------
            for g in range(NGROUPS):
                sl = slice(g * UG, (g + 1) * UG)
                for b in range(IMGS_PER_CORE):
                    s = st[b]
                    x1tR, y1tR, x2tR, y2tR = s["reps"]

                    def pv16(t):  # pred operand [P, M, UG], bcast on gt (middle)
                        return t[:, sl].unsqueeze(1).to_broadcast([P, M, UG])

                    ltx = grp.tile([P, M, UG], f16, tag="ltx", name="ltx")
                    rbx = grp.tile([P, M, UG], f16, tag="rbx", name="rbx")
                    lty = grp.tile([P, M, UG], f16, tag="lty", name="lty")
                    rby = grp.tile([P, M, UG], f16, tag="rby", name="rby")
                    ssum = grp.tile([P, M, UG], f32, tag="ssum", name="ssum")
                    rsc = grp.tile([P, M, UG], f32, tag="rsc", name="rsc")
                    rsc16 = grp.tile([P, M, UG], f16, tag="rsc16", name="rsc16")
                    # DVE: 4 minmax, f16 2x (both operands packed)
                    nc.vector.tensor_tensor(ltx[:], pv16(s["x1p"]), x1tR[:], op=Alu.max)
                    nc.vector.tensor_tensor(rbx[:], pv16(s["x2p"]), x2tR[:], op=Alu.min)
                    nc.vector.tensor_tensor(lty[:], pv16(s["y1p"]), y1tR[:], op=Alu.max)
                    nc.vector.tensor_tensor(rby[:], pv16(s["y2p"]), y2tR[:], op=Alu.min)
                    # Pool: ssum (bcast add, f32), dx, dy (f16) — in-place
                    nc.gpsimd.tensor_tensor(
                        ssum[:],
                        s["apred"][:, sl].unsqueeze(1).to_broadcast([P, M, UG]),
                        s["atB"][:].unsqueeze(2).to_broadcast([P, M, UG]),
                        op=Alu.add)
                    nc.gpsimd.tensor_tensor(rbx[:], rbx[:], ltx[:], op=Alu.subtract)
                    nc.gpsimd.tensor_tensor(rby[:], rby[:], lty[:], op=Alu.subtract)
                    # DVE: recip f32; Act: relu(dx)->ltx, rsc->f16
                    nc.vector.reciprocal_approx_fast(rsc[:], ssum[:])
                    nc.scalar.activation(ltx[:], rbx[:], Act.Relu)
                    nc.scalar.copy(rsc16[:], rsc[:])
                    # DVE: inter = relu(dx)*dy (f16 2x), q = inter*rsc16 (f16 2x)
                    nc.vector.tensor_tensor(lty[:], ltx[:], rby[:], op=Alu.mult)
                    qs = s["q"][:, :, sl]
                    nc.vector.tensor_tensor(qs, lty[:], rsc16[:], op=Alu.mult)
                    # DVE: running max accumulate
                    if g == 0:
                        nc.vector.tensor_copy(s["macc"][:], qs)
                    else:
                        nc.vector.tensor_tensor(s["macc"][:], s["macc"][:], qs,
                                                op=Alu.max)

            # ---------------- per-image: argmax recovery + tail ----------------
            for b in range(IMGS_PER_CORE):
                s = st[b]
                dbg = mkdbg(b)
                predsI = s["predsI"]
                tg = s["tg"]
                # ship q to DRAM for the later row gather
                nc.sync.dma_start(q_d[b].ap().rearrange("(p m) s -> p m s", p=P),
                                  s["q"][:])
                # final reduce -> m1 [P, M] f32 (f16 values, exactly embedded)
                m1 = sml.tile([P, M], f32, tag="m1", name="m1")
                nc.vector.tensor_reduce(m1[:], s["macc"][:], axis=X, op=Alu.max)
                # transpose m1 on PE -> [M, P] f32
                m1tp = psum.tile([M, P], f32, tag="m1tp", name="m1tp")
                nc.tensor.transpose(m1tp[:], m1[:], ident[:])
                m1t = sml.tile([M, P], f32, tag="m1t", name="m1t")
                nc.vector.tensor_copy(m1t[:], m1tp[:])
                # top-1 over partitions per GT
                mx8 = sml.tile([M, 8], f32, tag="mx8", name="mx8")
                pi8 = sml.tile([M, 8], u32, tag="pi8", name="pi8")
                nc.vector.max(mx8[:], m1t[:])
                nc.vector.max_index(pi8[:], mx8[:], m1t[:])

                # level-2: recover slot via row gather (f16 rows -> f32)
                rowoff = sml.tile([M, 2], u32, tag="rowoff", name="rowoff")
                nc.vector.tensor_scalar_mul(rowoff[:, 0:1], pi8[:, 0:1], M)
                nc.vector.tensor_tensor(rowoff[:, 0:1], rowoff[:, 0:1],
                                        iota_p64[:].bitcast(u32), op=Alu.add)
                qrow16 = sml.tile([M, SLOTS], f16, tag="qrow16", name="qrow16")
                nc.gpsimd.indirect_dma_start(
                    out=qrow16[:], out_offset=None,
                    in_=q_d[b].ap(),
                    in_offset=IndirectOffsetOnAxis(ap=rowoff[:, 0:1], axis=0))
                qrow = sml.tile([M, SLOTS], f32, tag="qrow", name="qrow")
                nc.vector.tensor_copy(qrow[:], qrow16[:])
                qx8 = sml.tile([M, 8], f32, tag="qx8", name="qx8")
                ci8 = sml.tile([M, 8], u32, tag="ci8", name="ci8")
                nc.vector.max(qx8[:], qrow[:])
                nc.vector.max_index(ci8[:], qx8[:], qrow[:])

                # n* = p* * SLOTS + c*  (u32), maxq = qx8[:,0:1]
                nstar = sml.tile([M, 1], u32, tag="nstar", name="nstar")
                nc.vector.tensor_scalar_mul(nstar[:], pi8[:, 0:1], SLOTS)
                nc.vector.tensor_tensor(nstar[:], nstar[:], ci8[:, 0:1], op=Alu.add)
                maxq = qx8[:, 0:1]

                # thr = maxq > 1/6
                thr = sml.tile([M, 1], f32, tag="thr", name="thr")
                nc.vector.tensor_scalar(thr[:], maxq, float(1.0 / 6.0), None,
                                        op0=Alu.is_gt)

                # dedup: ok[j] = thr[j] & !any(j'<j, thr & same n*)
                nstar_f = sml.tile([M, 1], f32, tag="nstar_f", name="nstar_f")
                nc.vector.tensor_copy(nstar_f[:], nstar[:])
                pair = sml.tile([M, 2], f32, tag="pair", name="pair")
                nc.vector.tensor_copy(pair[:, 0:1], nstar_f[:])
                nc.vector.tensor_copy(pair[:, 1:2], thr[:])
                pairT_ps = psum.tile([1, 2, M], f32, tag="pairT_ps", name="pairT_ps")
                nc.tensor.transpose(pairT_ps[:, 0], pair[:, 0:1], ident[:M, :M])
                nc.tensor.transpose(pairT_ps[:, 1], pair[:, 1:2], ident[:M, :M])
                pairT = sml.tile([1, 2, M], f32, tag="pairT", name="pairT")
                nc.vector.tensor_copy(pairT[:], pairT_ps[:])
                rowB = sml.tile([M, M, 2], f32, tag="rowB", name="rowB")
                ptb = psum.tile([M, M, 2], f32, tag="ptb", name="ptb")
                nc.tensor.matmul(ptb[:, :, 0], ones_row[:, :M], pairT[:, 0],
                                 start=True, stop=True)
                nc.tensor.matmul(ptb[:, :, 1], ones_row[:, :M], pairT[:, 1],
                                 start=True, stop=True)
                nc.scalar.copy(rowB[:], ptb[:])
                eq = sml.tile([M, M], f32, tag="eq", name="eq")
                nc.vector.tensor_scalar(eq[:], rowB[:, :, 0], nstar_f[:], None,
                                        op0=Alu.is_equal)
                nc.gpsimd.tensor_tensor(eq[:], eq[:], rowB[:, :, 1], op=Alu.mult)
                nc.vector.tensor_tensor(eq[:], eq[:], ltmask[:], op=Alu.mult)
                blocked = sml.tile([M, 1], f32, tag="blocked", name="blocked")
                nc.vector.tensor_reduce(blocked[:], eq[:], axis=X, op=Alu.max)
                ok = sml.tile([M, 1], f32, tag="ok", name="ok")
                nc.vector.tensor_scalar(ok[:], blocked[:], -1.0, 1.0,
                                        op0=Alu.mult, op1=Alu.add)
                nc.gpsimd.tensor_tensor(ok[:], ok[:], thr[:], op=Alu.mult)
                dbg("nstar", nstar[:], [M, 1], u32)
                dbg("thr", thr[:], [M, 1])
                dbg("ok", ok[:], [M, 1])

                # gather matched preds [M, 5]
                g5 = sml.tile([M, 5], f32, tag="g5", name="g5")
                nrow = sml.tile([M, 1], u32, tag="nrow", name="nrow")
                nc.vector.tensor_scalar_add(nrow[:], nstar[:], b * N)
                nc.gpsimd.indirect_dma_start(
                    out=g5[:], out_offset=None,
                    in_=preds_d.ap().rearrange("b n c -> (b n) c"),
                    in_offset=IndirectOffsetOnAxis(ap=nrow[:], axis=0))

                # ---------------- ciou on [M, 1] ----------------
                t1 = lambda tag: sml.tile([M, 1], f32, tag=tag, name=tag)
                gwc, ghc, gh2 = t1("gwc"), t1("ghc"), t1("gh2")
                nc.vector.tensor_scalar_max(gwc[:], g5[:, 2:3], 1e-4)
                nc.vector.tensor_scalar_max(ghc[:], g5[:, 3:4], 1e-4)
                px1, px2, py1, py2 = t1("px1"), t1("px2"), t1("py1"), t1("py2")
                nc.vector.tensor_scalar_mul(gh2[:], gwc[:], 0.5)
                nc.vector.tensor_sub(px1[:], g5[:, 0:1], gh2[:])
                nc.vector.tensor_add(px2[:], g5[:, 0:1], gh2[:])
                nc.vector.tensor_scalar_mul(gh2[:], ghc[:], 0.5)
                nc.vector.tensor_sub(py1[:], g5[:, 1:2], gh2[:])
                nc.vector.tensor_add(py2[:], g5[:, 1:2], gh2[:])
                tx1, ty1, tx2, ty2 = tg[:, 0:1], tg[:, 1:2], tg[:, 2:3], tg[:, 3:4]

                a1, a2, a3, a4 = t1("a1"), t1("a2"), t1("a3"), t1("a4")
                nc.vector.tensor_tensor(a1[:], px1[:], tx1, op=Alu.max)
                nc.vector.tensor_tensor(a2[:], px2[:], tx2, op=Alu.min)
                nc.vector.tensor_sub(a2[:], a2[:], a1[:])
                nc.vector.tensor_scalar_max(a2[:], a2[:], 0.0)
                nc.vector.tensor_tensor(a3[:], py1[:], ty1, op=Alu.max)
                nc.vector.tensor_tensor(a4[:], py2[:], ty2, op=Alu.min)
                nc.vector.tensor_sub(a4[:], a4[:], a3[:])
                nc.vector.tensor_scalar_max(a4[:], a4[:], 0.0)
                ginter = t1("ginter")
                nc.vector.tensor_tensor(ginter[:], a2[:], a4[:], op=Alu.mult)
                gwp, ghp, gwt, ght = t1("gwp"), t1("ghp"), t1("gwt"), t1("ght")
                nc.vector.tensor_sub(gwp[:], px2[:], px1[:])
                nc.vector.tensor_sub(ghp[:], py2[:], py1[:])
                nc.vector.tensor_sub(gwt[:], tx2, tx1)
                nc.vector.tensor_sub(ght[:], ty2, ty1)
                gu = t1("gu")
                nc.vector.tensor_tensor(gu[:], gwp[:], ghp[:], op=Alu.mult)
                nc.vector.tensor_tensor(a1[:], gwt[:], ght[:], op=Alu.mult)
                nc.vector.tensor_add(gu[:], gu[:], a1[:])
                nc.vector.tensor_sub(gu[:], gu[:], ginter[:])
                giou = t1("giou")
                nc.vector.tensor_scalar_add(gu[:], gu[:], float(EPS))
                nc.vector.reciprocal(gu[:], gu[:])
                nc.vector.tensor_tensor(giou[:], ginter[:], gu[:], op=Alu.mult)
                nc.vector.tensor_tensor(a1[:], px1[:], tx1, op=Alu.min)
                nc.vector.tensor_tensor(a2[:], px2[:], tx2, op=Alu.max)
                nc.vector.tensor_sub(a2[:], a2[:], a1[:])
                nc.vector.tensor_tensor(a2[:], a2[:], a2[:], op=Alu.mult)
                nc.vector.tensor_tensor(a3[:], py1[:], ty1, op=Alu.min)
                nc.vector.tensor_tensor(a4[:], py2[:], ty2, op=Alu.max)
                nc.vector.tensor_sub(a4[:], a4[:], a3[:])
                nc.vector.tensor_tensor(a4[:], a4[:], a4[:], op=Alu.mult)
                diag = t1("diag")
                nc.vector.tensor_add(diag[:], a2[:], a4[:])
                nc.vector.tensor_scalar_add(diag[:], diag[:], float(EPS))
                nc.vector.tensor_add(a1[:], px1[:], px2[:])
                nc.vector.tensor_sub(a1[:], a1[:], tx1)
                nc.vector.tensor_sub(a1[:], a1[:], tx2)
                nc.vector.tensor_tensor(a1[:], a1[:], a1[:], op=Alu.mult)
                nc.vector.tensor_add(a3[:], py1[:], py2[:])
                nc.vector.tensor_sub(a3[:], a3[:], ty1)
                nc.vector.tensor_sub(a3[:], a3[:], ty2)
                nc.vector.tensor_tensor(a3[:], a3[:], a3[:], op=Alu.mult)
                cent = t1("cent")
                nc.vector.tensor_add(cent[:], a1[:], a3[:])
                nc.vector.tensor_scalar_mul(cent[:], cent[:], 0.25)
                diou = t1("diou")
                nc.vector.reciprocal(diag[:], diag[:])
                nc.vector.tensor_tensor(diou[:], cent[:], diag[:], op=Alu.mult)
                nc.vector.tensor_sub(diou[:], diou[:], giou[:])
                nc.vector.tensor_scalar_add(diou[:], diou[:], 1.0)
                # v = 4/pi^2 * (atan(wt/ht) - atan(wp/hp))^2
                vv = t1("vv")
                rat = sml.tile([M, 2], f32, tag="rat", name="rat")
                big2 = sml.tile([M, 2], i32, tag="big2", name="big2")
                inv2 = sml.tile([M, 2], f32, tag="inv2", name="inv2")
                s2 = sml.tile([M, 2], f32, tag="s2", name="s2")
                ac2 = sml.tile([M, 2], f32, tag="ac2", name="ac2")
                nc.vector.reciprocal(rat[:, 0:1], ght[:])
                nc.vector.tensor_tensor(rat[:, 0:1], gwt[:], rat[:, 0:1], op=Alu.mult)
                nc.vector.reciprocal(rat[:, 1:2], ghp[:])
                nc.vector.tensor_tensor(rat[:, 1:2], gwp[:], rat[:, 1:2], op=Alu.mult)
                nc.vector.tensor_scalar(big2[:], rat[:], 1.0, None, op0=Alu.is_gt)
                nc.vector.reciprocal(inv2[:], rat[:])
                nc.vector.copy_predicated(rat[:], big2[:], inv2[:])
                nc.vector.tensor_tensor(s2[:], rat[:], rat[:], op=Alu.mult)
                nc.vector.tensor_scalar(ac2[:], s2[:], float(AT_POLY[0]),
                                        float(AT_POLY[1]), op0=Alu.mult, op1=Alu.add)
                for coef in AT_POLY[2:]:
                    nc.vector.tensor_tensor(ac2[:], ac2[:], s2[:], op=Alu.mult)
                    nc.vector.tensor_scalar_add(ac2[:], ac2[:], float(coef))
                nc.vector.tensor_tensor(ac2[:], ac2[:], rat[:], op=Alu.mult)
                nc.vector.tensor_scalar(inv2[:], ac2[:], -1.0, float(np.pi / 2),
                                        op0=Alu.mult, op1=Alu.add)
                nc.vector.copy_predicated(ac2[:], big2[:], inv2[:])
                nc.vector.tensor_sub(vv[:], ac2[:, 0:1], ac2[:, 1:2])
                nc.vector.tensor_tensor(vv[:], vv[:], vv[:], op=Alu.mult)
                nc.vector.tensor_scalar_mul(vv[:], vv[:], float(C_4PI2))
                # alpha = v / (1 - iou + v + eps)
                nc.vector.tensor_scalar(a1[:], giou[:], -1.0, float(1.0 + EPS),
                                        op0=Alu.mult, op1=Alu.add)
                nc.vector.tensor_add(a1[:], a1[:], vv[:])
                nc.vector.reciprocal(a1[:], a1[:])
                nc.vector.tensor_tensor(a1[:], a1[:], vv[:], op=Alu.mult)
                ciou = t1("ciou")
                nc.vector.tensor_tensor(ciou[:], a1[:], vv[:], op=Alu.mult)
                nc.vector.tensor_add(ciou[:], ciou[:], diou[:])
                dbg("ciou", ciou[:], [M, 1])
                # box_loss = sum(ciou*ok)/max(n_match,1)
                nc.vector.tensor_tensor(a1[:], ciou[:], ok[:], op=Alu.mult)
                bsum = sml.tile([1, 1], f32, tag="bsum", name="bsum")
                nmatch = sml.tile([1, 1], f32, tag="nmatch", name="nmatch")
                nc.gpsimd.tensor_reduce(bsum[:], a1[:], axis=C, op=Alu.add)
                nc.gpsimd.tensor_reduce(nmatch[:], ok[:], axis=C, op=Alu.add)
                nc.vector.tensor_scalar_max(nmatch[:], nmatch[:], 1.0)
                nc.vector.reciprocal(nmatch[:], nmatch[:])
                box_loss = sml.tile([1, 1], f32, tag="box_loss", name="box_loss")
                nc.vector.tensor_tensor(box_loss[:], bsum[:], nmatch[:], op=Alu.mult)

                # ---------------- focal loss ----------------
                def softplus_sigmoid(x_ap, shape, pool, pfx):
                    tl = lambda t: pool.tile(shape, f32, tag=pfx + t, name=pfx + t)
                    sg_, sp_, u_, w_, z_, e_ = (tl("sg"), tl("sp"), tl("u"),
                                                tl("w"), tl("z"), tl("e"))
                    nc.scalar.activation(e_[:], x_ap, Act.Exp, scale=-1.0)
                    nc.vector.tensor_scalar_add(e_[:], e_[:], 1.0)
                    nc.vector.reciprocal(sg_[:], e_[:])
                    nc.vector.tensor_scalar_mul(u_[:], x_ap, -1.0)
                    nc.vector.tensor_tensor(u_[:], u_[:], x_ap, op=Alu.max)
                    nc.scalar.activation(u_[:], u_[:], Act.Exp, scale=-1.0)
                    nc.vector.tensor_scalar_add(w_[:], u_[:], 1.0)
                    nc.vector.tensor_scalar(z_[:], u_[:], float(SP_SEED[0]),
                                            float(SP_SEED[1]), op0=Alu.mult,
                                            op1=Alu.add)
                    for coef in SP_SEED[2:]:
                        nc.vector.tensor_tensor(z_[:], z_[:], u_[:], op=Alu.mult)
                        nc.vector.tensor_scalar_add(z_[:], z_[:], float(coef))
                    nc.vector.tensor_tensor(z_[:], z_[:], u_[:], op=Alu.mult)
                    for _ in range(2):
                        nc.scalar.activation(e_[:], z_[:], Act.Exp, scale=-1.0)
                        nc.gpsimd.tensor_tensor(e_[:], w_[:], e_[:], op=Alu.mult)
                        nc.gpsimd.tensor_tensor(z_[:], z_[:], e_[:], op=Alu.add)
                        nc.vector.tensor_scalar_add(z_[:], z_[:], -1.0)
                    nc.scalar.activation(sp_[:], x_ap, Act.Relu)
                    nc.vector.tensor_add(sp_[:], sp_[:], z_[:])
                    return sg_, sp_

                conf = predsI[:, :, 4]
                sg, sp = softplus_sigmoid(conf, [P, SLOTS], der, "fb")
                f0 = der.tile([P, SLOTS], f32, tag="f0", name="f0")
                nc.gpsimd.tensor_tensor(f0[:], sg[:], sg[:], op=Alu.mult)
                nc.gpsimd.tensor_tensor(f0[:], f0[:], sp[:], op=Alu.mult)
                frow = sml.tile([P, 1], f32, tag="frow", name="frow")
                nc.vector.tensor_reduce(frow[:], f0[:], axis=X, op=Alu.add)
                fsum = sml.tile([1, 1], f32, tag="fsum", name="fsum")
                nc.gpsimd.tensor_reduce(fsum[:], frow[:], axis=C, op=Alu.add)
                # correction at matched preds: sum ok * (focal1 - focal0)
                xm = g5[:, 4:5]
                msg, msp = softplus_sigmoid(xm, [M, 1], sml, "fm")
                msn = t1("msn")
                nc.vector.tensor_sub(msn[:], msp[:], xm)
                mf0, mf1 = t1("mf0"), t1("mf1")
                nc.vector.tensor_tensor(mf0[:], msg[:], msg[:], op=Alu.mult)
                nc.vector.tensor_tensor(mf0[:], mf0[:], msp[:], op=Alu.mult)
                nc.vector.tensor_scalar_mul(mf0[:], mf0[:], 0.75)
                nc.vector.tensor_scalar(mf1[:], msg[:], -1.0, 1.0,
                                        op0=Alu.mult, op1=Alu.add)
                nc.vector.tensor_tensor(mf1[:], mf1[:], mf1[:], op=Alu.mult)
                nc.vector.tensor_tensor(mf1[:], mf1[:], msn[:], op=Alu.mult)
                nc.vector.tensor_scalar_mul(mf1[:], mf1[:], 0.25)
                nc.vector.tensor_sub(mf1[:], mf1[:], mf0[:])
                nc.vector.tensor_tensor(mf1[:], mf1[:], ok[:], op=Alu.mult)
                dsum = sml.tile([1, 1], f32, tag="dsum", name="dsum")
                nc.gpsimd.tensor_reduce(dsum[:], mf1[:], axis=C, op=Alu.add)

                # per_image = (0.75*fsum + dsum)/N + box_loss
                acc = sml.tile([1, 1], f32, tag="acc", name="acc")
                nc.vector.tensor_scalar_mul(acc[:], fsum[:], 0.75)
                nc.vector.tensor_add(acc[:], acc[:], dsum[:])
                nc.vector.tensor_scalar_mul(acc[:], acc[:], float(1.0 / N))
                nc.vector.tensor_add(acc[:], acc[:], box_loss[:])
                nc.sync.dma_start(out_d.ap()[b:b + 1], acc[:].rearrange("o m -> (o m)"))

    nc.compile()
    return nc


def _get_nc():
    if "nc" not in _cache:
        _cache["nc"] = _build()
    return _cache["nc"]


def kernel(preds: np.ndarray, targets: np.ndarray) -> np.ndarray:
    from concourse.bass_utils import run_bass_kernel_spmd

    nc = _get_nc()
    preds = np.ascontiguousarray(preds, dtype=np.float32)
    targets = np.ascontiguousarray(targets, dtype=np.float32)
    in_maps = []
    for c in range(N_CORES):
        s = c * IMGS_PER_CORE
        in_maps.append({"preds": preds[s:s + IMGS_PER_CORE],
                        "targets": targets[s:s + IMGS_PER_CORE]})
    res = run_bass_kernel_spmd(nc, in_maps, list(range(N_CORES)))
    per_image = np.concatenate([res.results[c]["out"] for c in range(N_CORES)])
    return np.float32(per_image.mean())


# revision 4
# speedup vs baseline: 1.4067x; 1.0511x over previous
"""Trainium2 Bass kernel for nn_DetectionLoss (B=16, N=25000, M=64).

Strategy (v3 — fp16 match pipeline, software-pipelined):
- Data-parallel: 8 cores x 2 images each. Host shards batch, kernel returns
  per-image losses [2] per core, host averages 16 values.
- Greedy match reformulated as per-GT argmax (exact; see baseline notes).
  Ranking uses q = inter/(area_p+area_t), monotone in iou; thr is q > 1/6.
- Match DECISIONS tolerate fp16 (numpy sim: 47/1024 idx flips, 1/1024 mask
  flips -> final rel err 3.5e-4; gate 2e-2). Loss tail stays exact f32.
- Bulk pairwise in fp16 for the DVE 2x_1p mode. Broadcast target operands
  (stride-0 last dim) would kill the mode, so each target channel is
  materialized once per image as a [P, M, UG] replicated tile (log-doubling
  TensorCopy at 4x) and reused by all 7 slot groups.
- Engines: DVE does minmax (f16 2x), recip (f32), inter/q (f16 2x), running
  max; Pool does ssum (bcast add) + dx/dy subs; Act does relu + rsc f32->f16.
- Emission is wave-skewed (2-wave software pipeline) per image; image 0's
  tail is sliced into pieces and interleaved into image 1's bulk waves so
  only image 1's tail is exposed at the end.
"""

import numpy as np

B, N, M = 16, 25000, 64
P = 128            # SBUF partitions; pred partition layout
SLOTS = 196        # slots per partition; P*SLOTS = 25088 >= N
IMGS_PER_CORE = 2
N_CORES = 8
UG = 28            # slots per bulk group
NGROUPS = SLOTS // UG  # 7

# partition 0..126 have all SLOTS valid; partition 127 has PAD_START.. invalid
PAD_PART = 127
PAD_START = N - PAD_PART * SLOTS   # 25000 - 24892 = 108

_cache = {}


def _build(debug_dumps=False):
    import concourse.bass as bass
    import concourse.bacc as bacc
    import concourse.mybir as mybir
    from concourse import tile
    from concourse.bass import IndirectOffsetOnAxis
    from concourse.masks import make_identity

    f32 = mybir.dt.float32
    f16 = mybir.dt.float16
    u32 = mybir.dt.uint32
    i32 = mybir.dt.int32
    Alu = mybir.AluOpType
    Act = mybir.ActivationFunctionType
    X = mybir.AxisListType.X
    C = mybir.AxisListType.C

    nc = bacc.Bacc("TRN2", target_bir_lowering=False, debug=False,
                   num_devices=N_CORES)

    preds_d = nc.dram_tensor("preds", [IMGS_PER_CORE, N, 5], f32, kind="ExternalInput")
    targets_d = nc.dram_tensor("targets", [IMGS_PER_CORE, M, 4], f32, kind="ExternalInput")
    out_d = nc.dram_tensor("out", [IMGS_PER_CORE], f32, kind="ExternalOutput")
    q_d = [nc.dram_tensor(f"q_scratch{b}", [P * M, SLOTS], f16)
           for b in range(IMGS_PER_CORE)]

    EPS = np.float32(1e-7)
    C_4PI2 = np.float32(4.0 / (np.pi ** 2))
    SP_SEED = [0.041064513, -0.156028432, 0.304672365, -0.496368282, 0.999887926]
    AT_POLY = [0.0030496317, -0.0168262157, 0.0438537714, -0.0759666934,
               0.1068136135, -0.1421318243, 0.1999371457, -0.3333312071,
               0.9999999881]

    with tile.TileContext(nc) as tc:
        with (
            tc.tile_pool(name="qpool", bufs=2) as qpool,
            tc.tile_pool(name="ppool", bufs=2) as ppool,
            tc.tile_pool(name="der", bufs=2) as der,
            tc.tile_pool(name="rep", bufs=2) as rep,
            tc.tile_pool(name="grp", bufs=3) as grp,
            tc.tile_pool(name="mac", bufs=2) as mac,
            tc.tile_pool(name="sml", bufs=2) as sml,
            tc.tile_pool(name="cst", bufs=1) as cst,
            tc.tile_pool(name="psum", bufs=2,
                         space=bass.MemorySpace.PSUM) as psum,
        ):
            # constants
            iota_p64 = cst.tile([M, 1], i32, tag="iota_p64")
            nc.gpsimd.iota(iota_p64[:], pattern=[[1, 1]], base=0, channel_multiplier=1)
            iota_f64 = cst.tile([M, M], i32, tag="iota_f64")
            nc.gpsimd.iota(iota_f64[:], pattern=[[1, M]], base=0, channel_multiplier=0)
            iota_p64f = cst.tile([M, 1], f32, tag="iota_p64f")
            nc.vector.tensor_copy(iota_p64f[:], iota_p64[:])
            iota_f64f = cst.tile([M, M], f32, tag="iota_f64f")
            nc.vector.tensor_copy(iota_f64f[:], iota_f64[:])
            ltmask = cst.tile([M, M], f32, tag="ltmask")
            nc.vector.tensor_scalar(ltmask[:], iota_f64f[:], iota_p64f[:], None,
                                    op0=Alu.is_lt)
            ones_row = cst.tile([1, P], f32, tag="ones_row")
            nc.gpsimd.memset(ones_row[:], 1.0)
            ident = cst.tile([P, P], f32, tag="ident")
            make_identity(nc, ident[:])

            st = [dict() for _ in range(IMGS_PER_CORE)]

            # ---------------- load DMAs for both images up front ----------
            for b in range(IMGS_PER_CORE):
                s = st[b]
                predsI = ppool.tile([P, SLOTS, 5], f32, tag="predsI", name="predsI")
                nc.gpsimd.memset(predsI[:, PAD_START:, 0:2], 50.0)
                nc.gpsimd.memset(predsI[:, PAD_START:, 2:4], 1e-4)
                nc.gpsimd.memset(predsI[:, PAD_START:, 4:5], -80.0)
                src = preds_d.ap()[b].rearrange("n c -> (n c)")
                nc.sync.dma_start(
                    predsI[:PAD_PART],
                    src[: PAD_PART * SLOTS * 5].rearrange("(p f) -> p f", p=PAD_PART)
                    .rearrange("p (s c) -> p s c", c=5))
                nc.sync.dma_start(
                    predsI[PAD_PART:, :PAD_START],
                    src[PAD_PART * SLOTS * 5:].rearrange("(p s c) -> p s c", p=1, c=5))
                s["predsI"] = predsI
                tg = sml.tile([M, 4], f32, tag="tg", name="tg")
                nc.sync.dma_start(tg[:], targets_d.ap()[b])
                trow = sml.tile([1, M, 4], f32, tag="trow", name="trow")
                nc.sync.dma_start(trow[:], targets_d.ap()[b].unsqueeze(0))
                s["tg"] = tg
                s["trow"] = trow

            # ---------------- prelude (derives, bcasts, reps) -------------
            def prelude(b):
                s = st[b]
                predsI = s["predsI"]
                wc = der.tile([P, SLOTS], f32, tag="wc", name="wc")
                hc = der.tile([P, SLOTS], f32, tag="hc", name="hc")
                half = der.tile([P, SLOTS], f32, tag="half", name="half")
                x1p = der.tile([P, SLOTS], f16, tag="x1p", name="x1p")
                x2p = der.tile([P, SLOTS], f16, tag="x2p", name="x2p")
                y1p = der.tile([P, SLOTS], f16, tag="y1p", name="y1p")
                y2p = der.tile([P, SLOTS], f16, tag="y2p", name="y2p")
                apred = der.tile([P, SLOTS], f32, tag="apred", name="apred")
                nc.vector.tensor_scalar_max(wc[:], predsI[:, :, 2], 1e-4)
                nc.vector.tensor_scalar_max(hc[:], predsI[:, :, 3], 1e-4)
                nc.vector.tensor_scalar_mul(half[:], wc[:], 0.5)
                nc.vector.tensor_tensor(x1p[:], predsI[:, :, 0], half[:],
                                        op=Alu.subtract)
                nc.vector.tensor_tensor(x2p[:], predsI[:, :, 0], half[:],
                                        op=Alu.add)
                nc.vector.tensor_scalar_mul(half[:], hc[:], 0.5)
                nc.vector.tensor_tensor(y1p[:], predsI[:, :, 1], half[:],
                                        op=Alu.subtract)
                nc.vector.tensor_tensor(y2p[:], predsI[:, :, 1], half[:],
                                        op=Alu.add)
                nc.vector.tensor_tensor(apred[:], wc[:], hc[:], op=Alu.mult)
                s.update(x1p=x1p, x2p=x2p, y1p=y1p, y2p=y2p, apred=apred)

                trow = s["trow"]
                atrow = sml.tile([1, M, 2], f32, tag="atrow", name="atrow")
                nc.vector.tensor_sub(atrow[:, :, 0], trow[:, :, 2], trow[:, :, 0])
                nc.vector.tensor_sub(atrow[:, :, 1], trow[:, :, 3], trow[:, :, 1])
                nc.vector.tensor_tensor(atrow[:, :, 0], atrow[:, :, 0],
                                        atrow[:, :, 1], op=Alu.mult)
                coord16 = []
                for ci in range(4):
                    pt = psum.tile([P, M], f32, tag="bcast_ps", name="bcast_ps")
                    nc.tensor.matmul(pt[:], ones_row[:], trow[:, :, ci],
                                     start=True, stop=True)
                    c16 = rep.tile([P, M], f16, tag=f"tb16_{ci}", name=f"tb16_{ci}")
                    nc.scalar.copy(c16[:], pt[:])
                    coord16.append(c16)
                pt = psum.tile([P, M], f32, tag="bcast_ps", name="bcast_ps")
                nc.tensor.matmul(pt[:], ones_row[:], atrow[:, :, 0],
                                 start=True, stop=True)
                atB = rep.tile([P, M], f32, tag="atB", name="atB")
                nc.scalar.copy(atB[:], pt[:])
                s["atB"] = atB
                reps = []
                for ci in range(4):
                    r = rep.tile([P, M, UG], f16, tag=f"rep_{ci}", name=f"rep_{ci}")
                    nc.vector.tensor_copy(r[:, :, 0:1],
                                          coord16[ci][:].unsqueeze(2))
                    k = 1
                    while k < UG:
                        step = min(k, UG - k)
                        nc.vector.tensor_copy(r[:, :, k:k + step], r[:, :, 0:step])
                        k += step
                    reps.append(r)
                s["reps"] = reps
                s["q"] = qpool.tile([P, M, SLOTS], f16, tag="q", name="q")
                s["macc"] = mac.tile([P, M, UG], f16, tag="macc", name="macc")
                s["grp"] = {}

            # ---------------- bulk wave stages ----------------------------
            def stage_mm(b, g):
                s = st[b]
                sl = slice(g * UG, (g + 1) * UG)
                x1tR, y1tR, x2tR, y2tR = s["reps"]

                def pv16(t):
                    return t[:, sl].unsqueeze(1).to_broadcast([P, M, UG])

                ltx = grp.tile([P, M, UG], f16, tag="ltx", name="ltx")
                rbx = grp.tile([P, M, UG], f16, tag="rbx", name="rbx")
                lty = grp.tile([P, M, UG], f16, tag="lty", name="lty")
                rby = grp.tile([P, M, UG], f16, tag="rby", name="rby")
                ssum = grp.tile([P, M, UG], f32, tag="ssum", name="ssum")
                rsc16 = grp.tile([P, M, UG], f16, tag="rsc16", name="rsc16")
                s["grp"][g] = (ltx, rbx, lty, rby, ssum, rsc16)
                nc.vector.tensor_tensor(ltx[:], pv16(s["x1p"]), x1tR[:], op=Alu.max)
                nc.vector.tensor_tensor(rbx[:], pv16(s["x2p"]), x2tR[:], op=Alu.min)
                nc.vector.tensor_tensor(lty[:], pv16(s["y1p"]), y1tR[:], op=Alu.max)
                nc.vector.tensor_tensor(rby[:], pv16(s["y2p"]), y2tR[:], op=Alu.min)
                # Pool: ssum (no bulk deps — keeps Pool ahead)
                nc.gpsimd.tensor_tensor(
                    ssum[:],
                    s["apred"][:, sl].unsqueeze(1).to_broadcast([P, M, UG]),
                    s["atB"][:].unsqueeze(2).to_broadcast([P, M, UG]),
                    op=Alu.add)

            def stage_mid(b, g):
                s = st[b]
                ltx, rbx, lty, rby, ssum, rsc16 = s["grp"][g]
                # Pool: dx, dy in place (dy on DVE for the last group: balance)
                nc.gpsimd.tensor_tensor(rbx[:], rbx[:], ltx[:], op=Alu.subtract)
                if g == NGROUPS - 1:
                    nc.vector.tensor_tensor(rby[:], rby[:], lty[:], op=Alu.subtract)
                else:
                    nc.gpsimd.tensor_tensor(rby[:], rby[:], lty[:], op=Alu.subtract)
                # DVE: recip f32 in place
                nc.vector.reciprocal_approx_fast(ssum[:], ssum[:])
                # Act: relu(dx) -> ltx, rsc -> f16
                nc.scalar.activation(ltx[:], rbx[:], Act.Relu)
                nc.scalar.copy(rsc16[:], ssum[:])

            def stage_fin(b, g):
                s = st[b]
                ltx, rbx, lty, rby, ssum, rsc16 = s["grp"][g]
                sl = slice(g * UG, (g + 1) * UG)
                # DVE: inter = relu(dx)*dy -> lty, q = inter*rsc16, macc
                nc.vector.tensor_tensor(lty[:], ltx[:], rby[:], op=Alu.mult)
                qs = s["q"][:, :, sl]
                nc.vector.tensor_tensor(qs, lty[:], rsc16[:], op=Alu.mult)
                if g == 0:
                    nc.vector.tensor_copy(s["macc"][:], qs)
                else:
                    nc.vector.tensor_tensor(s["macc"][:], s["macc"][:], qs,
                                            op=Alu.max)
                del s["grp"][g]

            # ---------------- focal bulk (per image, chunked emission) ----
            def focal_bulk_chunks(b):
                s = st[b]
                predsI = s["predsI"]
                tl = lambda t: der.tile([P, SLOTS], f32, tag="fb" + t, name="fb" + t)
                sg_, sp_, u_, w_, z_, e_ = (tl("sg"), tl("sp"), tl("u"),
                                            tl("w"), tl("z"), tl("e"))
                conf = predsI[:, :, 4]

                def c0():
                    nc.scalar.activation(e_[:], conf, Act.Exp, scale=-1.0)
                    nc.vector.tensor_scalar_add(e_[:], e_[:], 1.0)
                    nc.vector.reciprocal(sg_[:], e_[:])
                    nc.vector.tensor_scalar_mul(u_[:], conf, -1.0)
                    nc.vector.tensor_tensor(u_[:], u_[:], conf, op=Alu.max)
                    nc.scalar.activation(u_[:], u_[:], Act.Exp, scale=-1.0)
                    nc.vector.tensor_scalar_add(w_[:], u_[:], 1.0)

                def c1():
                    nc.vector.tensor_scalar(z_[:], u_[:], float(SP_SEED[0]),
                                            float(SP_SEED[1]), op0=Alu.mult,
                                            op1=Alu.add)
                    for coef in SP_SEED[2:]:
                        nc.vector.tensor_tensor(z_[:], z_[:], u_[:], op=Alu.mult)
                        nc.vector.tensor_scalar_add(z_[:], z_[:], float(coef))
                    nc.vector.tensor_tensor(z_[:], z_[:], u_[:], op=Alu.mult)

                def c2():
                    for _ in range(2):
                        nc.scalar.activation(e_[:], z_[:], Act.Exp, scale=-1.0)
                        nc.vector.tensor_tensor(e_[:], w_[:], e_[:], op=Alu.mult)
                        nc.vector.tensor_tensor(z_[:], z_[:], e_[:], op=Alu.add)
                        nc.vector.tensor_scalar_add(z_[:], z_[:], -1.0)

                def c3():
                    nc.scalar.activation(sp_[:], conf, Act.Relu)
                    nc.vector.tensor_add(sp_[:], sp_[:], z_[:])
                    f0 = u_
                    nc.vector.tensor_tensor(f0[:], sg_[:], sg_[:], op=Alu.mult)
                    nc.vector.tensor_tensor(f0[:], f0[:], sp_[:], op=Alu.mult)
                    frow = sml.tile([P, 1], f32, tag="frow", name="frow")
                    nc.vector.tensor_reduce(frow[:], f0[:], axis=X, op=Alu.add)
                    fsum = sml.tile([1, 1], f32, tag="fsum", name="fsum")
                    nc.gpsimd.tensor_reduce(fsum[:], frow[:], axis=C, op=Alu.add)
                    s["fsum"] = fsum
                    s["msg_in"] = (sg_, sp_)  # keep tiles alive

                return [c0, c1, c2, c3]

            # ---------------- tail pieces (per image) ---------------------
            def tail_pieces(b):
                s = st[b]
                tg = s["tg"]
                h = {}
                t1 = lambda tag: sml.tile([M, 1], f32, tag=tag, name=tag)

                def p0():
                    nc.sync.dma_start(
                        q_d[b].ap().rearrange("(p m) s -> p m s", p=P), s["q"][:])
                    m1 = sml.tile([P, M], f32, tag="m1", name="m1")
                    nc.vector.tensor_reduce(m1[:], s["macc"][:], axis=X, op=Alu.max)
                    m1tp = psum.tile([M, P], f32, tag="m1tp", name="m1tp")
                    nc.tensor.transpose(m1tp[:], m1[:], ident[:])
                    m1t = sml.tile([M, P], f32, tag="m1t", name="m1t")
                    nc.vector.tensor_copy(m1t[:], m1tp[:])
                    mx8 = sml.tile([M, 8], f32, tag="mx8", name="mx8")
                    pi8 = sml.tile([M, 8], u32, tag="pi8", name="pi8")
                    nc.vector.max(mx8[:], m1t[:])
                    nc.vector.max_index(pi8[:], mx8[:], m1t[:])
                    rowoff = sml.tile([M, 2], u32, tag="rowoff", name="rowoff")
                    nc.vector.tensor_scalar_mul(rowoff[:, 0:1], pi8[:, 0:1], M)
                    nc.vector.tensor_tensor(rowoff[:, 0:1], rowoff[:, 0:1],
                                            iota_p64[:].bitcast(u32), op=Alu.add)
                    qrow16 = sml.tile([M, SLOTS], f16, tag="qrow16", name="qrow16")
                    nc.gpsimd.indirect_dma_start(
                        out=qrow16[:], out_offset=None,
                        in_=q_d[b].ap(),
                        in_offset=IndirectOffsetOnAxis(ap=rowoff[:, 0:1], axis=0))
                    h.update(pi8=pi8, qrow16=qrow16)

                def p1():
                    qrow = sml.tile([M, SLOTS], f32, tag="qrow", name="qrow")
                    nc.vector.tensor_copy(qrow[:], h["qrow16"][:])
                    qx8 = sml.tile([M, 8], f32, tag="qx8", name="qx8")
                    ci8 = sml.tile([M, 8], u32, tag="ci8", name="ci8")
                    nc.vector.max(qx8[:], qrow[:])
                    nc.vector.max_index(ci8[:], qx8[:], qrow[:])
                    nstar = sml.tile([M, 1], u32, tag="nstar", name="nstar")
                    nc.vector.tensor_scalar_mul(nstar[:], h["pi8"][:, 0:1], SLOTS)
                    nc.vector.tensor_tensor(nstar[:], nstar[:], ci8[:, 0:1],
                                            op=Alu.add)
                    thr = t1("thr")
                    nc.vector.tensor_scalar(thr[:], qx8[:, 0:1], float(1.0 / 6.0),
                                            None, op0=Alu.is_gt)
                    # start g5 gather early (independent of dedup)
                    g5 = sml.tile([M, 5], f32, tag="g5", name="g5")
                    nrow = sml.tile([M, 1], u32, tag="nrow", name="nrow")
                    nc.vector.tensor_scalar_add(nrow[:], nstar[:], b * N)
                    nc.gpsimd.indirect_dma_start(
                        out=g5[:], out_offset=None,
                        in_=preds_d.ap().rearrange("b n c -> (b n) c"),
                        in_offset=IndirectOffsetOnAxis(ap=nrow[:], axis=0))
                    h.update(nstar=nstar, thr=thr, g5=g5)

                def p2():
                    nstar, thr = h["nstar"], h["thr"]
                    nstar_f = sml.tile([M, 1], f32, tag="nstar_f", name="nstar_f")
                    nc.vector.tensor_copy(nstar_f[:], nstar[:])
                    pair = sml.tile([M, 2], f32, tag="pair", name="pair")
                    nc.vector.tensor_copy(pair[:, 0:1], nstar_f[:])
                    nc.vector.tensor_copy(pair[:, 1:2], thr[:])
                    pairT_ps = psum.tile([1, 2, M], f32, tag="pairT_ps",
                                         name="pairT_ps")
                    nc.tensor.transpose(pairT_ps[:, 0], pair[:, 0:1], ident[:M, :M])
                    nc.tensor.transpose(pairT_ps[:, 1], pair[:, 1:2], ident[:M, :M])
                    pairT = sml.tile([1, 2, M], f32, tag="pairT", name="pairT")
                    nc.vector.tensor_copy(pairT[:], pairT_ps[:])
                    rowB = sml.tile([M, M, 2], f32, tag="rowB", name="rowB")
                    ptb = psum.tile([M, M, 2], f32, tag="ptb", name="ptb")
                    nc.tensor.matmul(ptb[:, :, 0], ones_row[:, :M], pairT[:, 0],
                                     start=True, stop=True)
                    nc.tensor.matmul(ptb[:, :, 1], ones_row[:, :M], pairT[:, 1],
                                     start=True, stop=True)
                    nc.scalar.copy(rowB[:], ptb[:])
                    eq = sml.tile([M, M], f32, tag="eq", name="eq")
                    nc.vector.tensor_scalar(eq[:], rowB[:, :, 0], nstar_f[:], None,
                                            op0=Alu.is_equal)
                    nc.gpsimd.tensor_tensor(eq[:], eq[:], rowB[:, :, 1], op=Alu.mult)
                    nc.vector.tensor_tensor(eq[:], eq[:], ltmask[:], op=Alu.mult)
                    blocked = sml.tile([M, 1], f32, tag="blocked", name="blocked")
                    nc.vector.tensor_reduce(blocked[:], eq[:], axis=X, op=Alu.max)
                    ok = sml.tile([M, 1], f32, tag="ok", name="ok")
                    nc.vector.tensor_scalar(ok[:], blocked[:], -1.0, 1.0,
                                            op0=Alu.mult, op1=Alu.add)
                    nc.gpsimd.tensor_tensor(ok[:], ok[:], thr[:], op=Alu.mult)
                    h["ok"] = ok

                def p3():
                    g5 = h["g5"]
                    gwc, ghc, gh2 = t1("gwc"), t1("ghc"), t1("gh2")
                    nc.vector.tensor_scalar_max(gwc[:], g5[:, 2:3], 1e-4)
                    nc.vector.tensor_scalar_max(ghc[:], g5[:, 3:4], 1e-4)
                    px1, px2, py1, py2 = t1("px1"), t1("px2"), t1("py1"), t1("py2")
                    nc.vector.tensor_scalar_mul(gh2[:], gwc[:], 0.5)
                    nc.vector.tensor_sub(px1[:], g5[:, 0:1], gh2[:])
                    nc.vector.tensor_add(px2[:], g5[:, 0:1], gh2[:])
                    nc.vector.tensor_scalar_mul(gh2[:], ghc[:], 0.5)
                    nc.vector.tensor_sub(py1[:], g5[:, 1:2], gh2[:])
                    nc.vector.tensor_add(py2[:], g5[:, 1:2], gh2[:])
                    tx1, ty1 = tg[:, 0:1], tg[:, 1:2]
                    tx2, ty2 = tg[:, 2:3], tg[:, 3:4]
                    a1, a2, a3, a4 = t1("a1"), t1("a2"), t1("a3"), t1("a4")
                    nc.vector.tensor_tensor(a1[:], px1[:], tx1, op=Alu.max)
                    nc.vector.tensor_tensor(a2[:], px2[:], tx2, op=Alu.min)
                    nc.vector.tensor_sub(a2[:], a2[:], a1[:])
                    nc.vector.tensor_scalar_max(a2[:], a2[:], 0.0)
                    nc.vector.tensor_tensor(a3[:], py1[:], ty1, op=Alu.max)
                    nc.vector.tensor_tensor(a4[:], py2[:], ty2, op=Alu.min)
                    nc.vector.tensor_sub(a4[:], a4[:], a3[:])
                    nc.vector.tensor_scalar_max(a4[:], a4[:], 0.0)
                    ginter = t1("ginter")
                    nc.vector.tensor_tensor(ginter[:], a2[:], a4[:], op=Alu.mult)
                    gwp, ghp, gwt, ght = t1("gwp"), t1("ghp"), t1("gwt"), t1("ght")
                    nc.vector.tensor_sub(gwp[:], px2[:], px1[:])
                    nc.vector.tensor_sub(ghp[:], py2[:], py1[:])
                    nc.vector.tensor_sub(gwt[:], tx2, tx1)
                    nc.vector.tensor_sub(ght[:], ty2, ty1)
                    gu = t1("gu")
                    nc.vector.tensor_tensor(gu[:], gwp[:], ghp[:], op=Alu.mult)
                    nc.vector.tensor_tensor(a1[:], gwt[:], ght[:], op=Alu.mult)
                    nc.vector.tensor_add(gu[:], gu[:], a1[:])
                    nc.vector.tensor_sub(gu[:], gu[:], ginter[:])
                    giou = t1("giou")
                    nc.vector.tensor_scalar_add(gu[:], gu[:], float(EPS))
                    nc.vector.reciprocal(gu[:], gu[:])
                    nc.vector.tensor_tensor(giou[:], ginter[:], gu[:], op=Alu.mult)
                    h.update(px1=px1, px2=px2, py1=py1, py2=py2, giou=giou,
                             gwp=gwp, ghp=ghp, gwt=gwt, ght=ght,
                             a1=a1, a2=a2, a3=a3, a4=a4)

                def p4():
                    px1, px2 = h["px1"], h["px2"]
                    py1, py2 = h["py1"], h["py2"]
                    a1, a2, a3, a4 = h["a1"], h["a2"], h["a3"], h["a4"]
                    giou = h["giou"]
                    gwp, ghp, gwt, ght = h["gwp"], h["ghp"], h["gwt"], h["ght"]
                    tx1, ty1 = tg[:, 0:1], tg[:, 1:2]
                    tx2, ty2 = tg[:, 2:3], tg[:, 3:4]
                    nc.vector.tensor_tensor(a1[:], px1[:], tx1, op=Alu.min)
                    nc.vector.tensor_tensor(a2[:], px2[:], tx2, op=Alu.max)
                    nc.vector.tensor_sub(a2[:], a2[:], a1[:])
                    nc.vector.tensor_tensor(a2[:], a2[:], a2[:], op=Alu.mult)
                    nc.vector.tensor_tensor(a3[:], py1[:], ty1, op=Alu.min)
                    nc.vector.tensor_tensor(a4[:], py2[:], ty2, op=Alu.max)
                    nc.vector.tensor_sub(a4[:], a4[:], a3[:])
                    nc.vector.tensor_tensor(a4[:], a4[:], a4[:], op=Alu.mult)
                    diag = t1("diag")
                    nc.vector.tensor_add(diag[:], a2[:], a4[:])
                    nc.vector.tensor_scalar_add(diag[:], diag[:], float(EPS))
                    nc.vector.tensor_add(a1[:], px1[:], px2[:])
                    nc.vector.tensor_sub(a1[:], a1[:], tx1)
                    nc.vector.tensor_sub(a1[:], a1[:], tx2)
                    nc.vector.tensor_tensor(a1[:], a1[:], a1[:], op=Alu.mult)
                    nc.vector.tensor_add(a3[:], py1[:], py2[:])
                    nc.vector.tensor_sub(a3[:], a3[:], ty1)
                    nc.vector.tensor_sub(a3[:], a3[:], ty2)
                    nc.vector.tensor_tensor(a3[:], a3[:], a3[:], op=Alu.mult)
                    cent = t1("cent")
                    nc.vector.tensor_add(cent[:], a1[:], a3[:])
                    nc.vector.tensor_scalar_mul(cent[:], cent[:], 0.25)
                    diou = t1("diou")
                    nc.vector.reciprocal(diag[:], diag[:])
                    nc.vector.tensor_tensor(diou[:], cent[:], diag[:], op=Alu.mult)
                    nc.vector.tensor_sub(diou[:], diou[:], giou[:])
                    nc.vector.tensor_scalar_add(diou[:], diou[:], 1.0)
                    vv = t1("vv")
                    rat = sml.tile([M, 2], f32, tag="rat", name="rat")
                    big2 = sml.tile([M, 2], i32, tag="big2", name="big2")
                    inv2 = sml.tile([M, 2], f32, tag="inv2", name="inv2")
                    s2 = sml.tile([M, 2], f32, tag="s2", name="s2")
                    ac2 = sml.tile([M, 2], f32, tag="ac2", name="ac2")
                    nc.vector.reciprocal(rat[:, 0:1], ght[:])
                    nc.vector.tensor_tensor(rat[:, 0:1], gwt[:], rat[:, 0:1],
                                            op=Alu.mult)
                    nc.vector.reciprocal(rat[:, 1:2], ghp[:])
                    nc.vector.tensor_tensor(rat[:, 1:2], gwp[:], rat[:, 1:2],
                                            op=Alu.mult)
                    nc.vector.tensor_scalar(big2[:], rat[:], 1.0, None, op0=Alu.is_gt)
                    nc.vector.reciprocal(inv2[:], rat[:])
                    nc.vector.copy_predicated(rat[:], big2[:], inv2[:])
                    nc.vector.tensor_tensor(s2[:], rat[:], rat[:], op=Alu.mult)
                    nc.vector.tensor_scalar(ac2[:], s2[:], float(AT_POLY[0]),
                                            float(AT_POLY[1]), op0=Alu.mult,
                                            op1=Alu.add)
                    for coef in AT_POLY[2:]:
                        nc.vector.tensor_tensor(ac2[:], ac2[:], s2[:], op=Alu.mult)
                        nc.vector.tensor_scalar_add(ac2[:], ac2[:], float(coef))
                    nc.vector.tensor_tensor(ac2[:], ac2[:], rat[:], op=Alu.mult)
                    nc.vector.tensor_scalar(inv2[:], ac2[:], -1.0, float(np.pi / 2),
                                            op0=Alu.mult, op1=Alu.add)
                    nc.vector.copy_predicated(ac2[:], big2[:], inv2[:])
                    nc.vector.tensor_sub(vv[:], ac2[:, 0:1], ac2[:, 1:2])
                    nc.vector.tensor_tensor(vv[:], vv[:], vv[:], op=Alu.mult)
                    nc.vector.tensor_scalar_mul(vv[:], vv[:], float(C_4PI2))
                    nc.vector.tensor_scalar(a1[:], giou[:], -1.0, float(1.0 + EPS),
                                            op0=Alu.mult, op1=Alu.add)
                    nc.vector.tensor_add(a1[:], a1[:], vv[:])
                    nc.vector.reciprocal(a1[:], a1[:])
                    nc.vector.tensor_tensor(a1[:], a1[:], vv[:], op=Alu.mult)
                    ciou = t1("ciou")
                    nc.vector.tensor_tensor(ciou[:], a1[:], vv[:], op=Alu.mult)
                    nc.vector.tensor_add(ciou[:], ciou[:], diou[:])
                    ok = h["ok"]
                    nc.vector.tensor_tensor(a1[:], ciou[:], ok[:], op=Alu.mult)
                    bsum = sml.tile([1, 1], f32, tag="bsum", name="bsum")
                    nmatch = sml.tile([1, 1], f32, tag="nmatch", name="nmatch")
                    nc.gpsimd.tensor_reduce(bsum[:], a1[:], axis=C, op=Alu.add)
                    nc.gpsimd.tensor_reduce(nmatch[:], ok[:], axis=C, op=Alu.add)
                    nc.vector.tensor_scalar_max(nmatch[:], nmatch[:], 1.0)
                    nc.vector.reciprocal(nmatch[:], nmatch[:])
                    box_loss = sml.tile([1, 1], f32, tag="box_loss", name="box_loss")
                    nc.vector.tensor_tensor(box_loss[:], bsum[:], nmatch[:],
                                            op=Alu.mult)
                    h["box_loss"] = box_loss

                def p5():
                    s_ = st[b]
                    g5, ok = h["g5"], h["ok"]
                    xm = g5[:, 4:5]
                    tl = lambda t: sml.tile([M, 1], f32, tag="fm" + t, name="fm" + t)
                    sg_, sp_, u_, w_, z_, e_ = (tl("sg"), tl("sp"), tl("u"),
                                                tl("w"), tl("z"), tl("e"))
                    nc.scalar.activation(e_[:], xm, Act.Exp, scale=-1.0)
                    nc.vector.tensor_scalar_add(e_[:], e_[:], 1.0)
                    nc.vector.reciprocal(sg_[:], e_[:])
                    nc.vector.tensor_scalar_mul(u_[:], xm, -1.0)
                    nc.vector.tensor_tensor(u_[:], u_[:], xm, op=Alu.max)
                    nc.scalar.activation(u_[:], u_[:], Act.Exp, scale=-1.0)
                    nc.vector.tensor_scalar_add(w_[:], u_[:], 1.0)
                    nc.vector.tensor_scalar(z_[:], u_[:], float(SP_SEED[0]),
                                            float(SP_SEED[1]), op0=Alu.mult,
                                            op1=Alu.add)
                    for coef in SP_SEED[2:]:
                        nc.vector.tensor_tensor(z_[:], z_[:], u_[:], op=Alu.mult)
                        nc.vector.tensor_scalar_add(z_[:], z_[:], float(coef))
                    nc.vector.tensor_tensor(z_[:], z_[:], u_[:], op=Alu.mult)
                    for _ in range(2):
                        nc.scalar.activation(e_[:], z_[:], Act.Exp, scale=-1.0)
                        nc.vector.tensor_tensor(e_[:], w_[:], e_[:], op=Alu.mult)
                        nc.vector.tensor_tensor(z_[:], z_[:], e_[:], op=Alu.add)
                        nc.vector.tensor_scalar_add(z_[:], z_[:], -1.0)
                    nc.scalar.activation(sp_[:], xm, Act.Relu)
                    nc.vector.tensor_add(sp_[:], sp_[:], z_[:])
                    msn = t1("msn")
                    nc.vector.tensor_sub(msn[:], sp_[:], xm)
                    mf0, mf1 = t1("mf0"), t1("mf1")
                    nc.vector.tensor_tensor(mf0[:], sg_[:], sg_[:], op=Alu.mult)
                    nc.vector.tensor_tensor(mf0[:], mf0[:], sp_[:], op=Alu.mult)
                    nc.vector.tensor_scalar_mul(mf0[:], mf0[:], 0.75)
                    nc.vector.tensor_scalar(mf1[:], sg_[:], -1.0, 1.0,
                                            op0=Alu.mult, op1=Alu.add)
                    nc.vector.tensor_tensor(mf1[:], mf1[:], mf1[:], op=Alu.mult)
                    nc.vector.tensor_tensor(mf1[:], mf1[:], msn[:], op=Alu.mult)
                    nc.vector.tensor_scalar_mul(mf1[:], mf1[:], 0.25)
                    nc.vector.tensor_sub(mf1[:], mf1[:], mf0[:])
                    nc.vector.tensor_tensor(mf1[:], mf1[:], ok[:], op=Alu.mult)
                    dsum = sml.tile([1, 1], f32, tag="dsum", name="dsum")
                    nc.gpsimd.tensor_reduce(dsum[:], mf1[:], axis=C, op=Alu.add)
                    acc = sml.tile([1, 1], f32, tag="acc", name="acc")
                    nc.vector.tensor_scalar_mul(acc[:], s_["fsum"][:], 0.75)
                    nc.vector.tensor_add(acc[:], acc[:], dsum[:])
                    nc.vector.tensor_scalar_mul(acc[:], acc[:], float(1.0 / N))
                    nc.vector.tensor_add(acc[:], acc[:], h["box_loss"][:])
                    nc.sync.dma_start(out_d.ap()[b:b + 1],
                                      acc[:].rearrange("o m -> (o m)"))

                return [p0, p1, p2, p3, p4, p5]

            # ---------------- emission schedule ---------------------------
            # image 0: prelude, bulk waves (2-wave pipeline) + focal chunks
            prelude(0)
            fb0 = focal_bulk_chunks(0)
            for w in range(NGROUPS + 2):
                if w < NGROUPS:
                    stage_mm(0, w)
                if w == 1:
                    prelude(1)      # fills DVE while Pool grinds image 0
                if 1 <= w <= NGROUPS:
                    stage_mid(0, w - 1)
                if w >= 2:
                    stage_fin(0, w - 2)
                if 2 <= w - 0 <= 5 and (w - 2) < len(fb0):
                    fb0[w - 2]()
            # image 1 bulk + image 0 tail pieces interleaved
            tp0 = tail_pieces(0)
            fb1 = focal_bulk_chunks(1)
            for w in range(NGROUPS + 2):
                if w < NGROUPS:
                    stage_mm(1, w)
                if 1 <= w <= NGROUPS:
                    stage_mid(1, w - 1)
                if w >= 2:
                    stage_fin(1, w - 2)
                if w < len(tp0):
                    tp0[w]()
                if 2 <= w and (w - 2) < len(fb1):
                    fb1[w - 2]()
            # image 1 tail
            for p in tail_pieces(1):
                p()

    nc.compile()
    return nc


def _get_nc():
    if "nc" not in _cache:
        _cache["nc"] = _build()
    return _cache["nc"]


def kernel(preds: np.ndarray, targets: np.ndarray) -> np.ndarray:
    from concourse.bass_utils import run_bass_kernel_spmd

    nc = _get_nc()
    preds = np.ascontiguousarray(preds, dtype=np.float32)
    targets = np.ascontiguousarray(targets, dtype=np.float32)
    in_maps = []
    for c in range(N_CORES):
        s = c * IMGS_PER_CORE
        in_maps.append({"preds": preds[s:s + IMGS_PER_CORE],
                        "targets": targets[s:s + IMGS_PER_CORE]})
    res = run_bass_kernel_spmd(nc, in_maps, list(range(N_CORES)))
    per_image = np.concatenate([res.results[c]["out"] for c in range(N_CORES)])
    return np.float32(per_image.mean())
